# revision 22
# baseline (speedup 1.0000x reference)
"""AudioFinder Trainium2 kernel.

Data parallel over batch: 16 samples -> 8 cores x 2 samples.

Per-core pipeline (bf16 matmuls / f32 psum, both samples interleaved
layer-by-layer so one sample's matmuls fill the other's pipeline-latency
bubbles on the in-order engine queues):
  1. Both query encoders (T=2048 -> 504), layers interleaved; v =
     w_lin @ enc_q; the tiled-x4 + pad row [1,2048] is bounced through
     DRAM into [128,16] (t = p + 128*b) while the search encoders run.
  2. Both search encoders (T=8192 -> 2040), layers interleaved, with
     the f0/f1 head fused into layer 3 and VQ 128-t blocks emitted as
     separate scheduler units paced between conv chunks so the DVE
     max-reduces never back up the PE's PSUM banks.
  3. VQ per 128-t block: three bf16 matmuls (enc block stationary)
     against epk3's three NK-column blocks, DVE max-reduce each:
     scores s[t,k] = enc_s[t]@emb[k] - |emb[k]|^2/2 (enc rows 80/81
     const 1.0; epk3 row 80 carries -|e|^2/2 shared, row 81 carries
     0 / ew0/BIG / ew1/BIG with ew = emb @ w_lin.T):
       u_j[t] = max_k (s[t,k] + ew[k,j]/BIG),  m[t] = max_k s[t,k]
       => (u_j - m)*BIG = ew[argmax_k s, j]   (fp32-psum exact)
  4. z = (u-m)*BIG + vt in [128,16]; max over free dim on DVE, across
     partitions on Pool; out = tanh(max z + b_lin).

All 1x1-conv biases are folded (host-side) into the next layer's tap
biases / f0's bias, so the residual writers are plain psum+residual
adds on DVE and layer-0's writer is a pair of ACT copies.  Every
matmul keeps a 128-partition stationary operand (zero rows beyond 80)
so FWL stays enabled; x2/xf ring rows 80-127 are zeroed once at start.
Gated tanh*sigmoid on ACT, gate product on Pool.
"""

import numpy as np
import ml_dtypes

import concourse.bacc as bacc
import concourse.mybir as mybir
import concourse.tile as tile
from concourse.bass_utils import run_bass_kernel_spmd

F32 = mybir.dt.float32
BF16 = mybir.dt.bfloat16
AF = mybir.ActivationFunctionType
OP = mybir.AluOpType
AX = mybir.AxisListType

NCORES = 8
SPC = 2          # samples per core
C = 80
NK = 512         # codebook size
BIG = 1024.0
NEG = -1e30
CH = 512         # chunk (free-dim) size

# layer geometry
GEO_SEARCH = dict(T0h=4096, T1=4095, E1=2048, O1=2047, T2=2046, T3=2043, T4=2040)
GEO_QUERY = dict(T0h=1024, T1=1023, E1=512, O1=511, T2=510, T3=507, T4=504)

# wpack layout: layer-major [a_i(4 taps), g_i(4 taps), w1x1_i] x 4, f0, f1
LBLK = 720  # 4*80 + 4*80 + 80

def _w_off(kind, i, j=0):
    if kind == "a":
        return LBLK * i + C * j
    if kind == "g":
        return LBLK * i + 320 + C * j
    if kind == "1":
        return LBLK * i + 640
    if kind == "f0":
        return 2880
    if kind == "f1":
        return 2960
    raise KeyError(kind)


M_F1 = 82  # f1 conv emits 80 real channels + two const-1 channels


WPACK_COLS = 3042
# bias pack columns: ba0..3, bg0..3, b10..3, bf0, bf1
def _b_off(kind, i=0):
    return {"a": i, "g": 4 + i, "1": 8 + i, "f0": 12, "f1": 13}[kind]


def _build():
    nc = bacc.Bacc("TRN2", target_bir_lowering=False, debug=False,
                   num_devices=NCORES)
    d_se = nc.dram_tensor("se", [SPC, 128, 4096], BF16, kind="ExternalInput")
    d_so = nc.dram_tensor("so", [SPC, 128, 4096], BF16, kind="ExternalInput")
    d_qe = nc.dram_tensor("qe", [SPC, 128, 1024], BF16, kind="ExternalInput")
    d_qo = nc.dram_tensor("qo", [SPC, 128, 1024], BF16, kind="ExternalInput")
    d_wpk = nc.dram_tensor("wpk", [128, WPACK_COLS], BF16, kind="ExternalInput")
    d_bpk = nc.dram_tensor("bpk", [M_F1, 14], F32, kind="ExternalInput")
    d_epk3 = nc.dram_tensor("epk3", [128, 3 * NK], BF16, kind="ExternalInput")
    d_z = nc.dram_tensor("z48", [48, 4104], BF16, kind="ExternalInput")
    d_ztail = nc.dram_tensor("ztail", [2, 32], F32, kind="ExternalInput")
    d_wlt = nc.dram_tensor("wlt", [128, 2], BF16, kind="ExternalInput")
    d_blt = nc.dram_tensor("blt", [1, 4], F32, kind="ExternalInput")
    d_out = nc.dram_tensor("out", [1, 4], F32, kind="ExternalOutput")
    d_zbuf = nc.dram_tensor("zbuf", [2 * SPC, 2048], F32)

    with tile.TileContext(nc) as tc:
        with (
            tc.tile_pool(name="sb", bufs=1) as sb,
            tc.tile_pool(name="ps", bufs=2, space="PSUM") as ps,
        ):
            # ---- startup DMAs, ordered so layer-0 work can start early ----
            wpk = sb.tile([128, WPACK_COLS], BF16, tag="wpk")
            nc.sync.dma_start(wpk[:, :LBLK], d_wpk[:, :LBLK])
            bpk = sb.tile([M_F1, 14], F32, tag="bpk")
            nc.sync.dma_start(bpk[:], d_bpk[:])
            # zero rows 80-127 of the x2/xf ring buffers once, so the
            # 1x1/f1 matmuls can use 128-partition operands (FWL on)
            for _ in range(4):
                x2t = sb.tile([128, CH], BF16, tag="x2", bufs=4)
                nc.vector.memset(x2t[64:, :], 0.0)
                xft = sb.tile([128, CH], BF16, tag="xf", bufs=4)
                nc.vector.memset(xft[64:, :], 0.0)

            def wsl(kind, i, j=0, rows=128):
                off = _w_off(kind, i, j)
                return wpk[:rows, off:off + C]

            def bap(kind, i=0):
                o = _b_off(kind, i)
                n = M_F1 if kind == "f1" else C
                return bpk[:n, o:o + 1]

            def wide_chunk_segs(i, taps, T_out, write_out, c0):
                """One CH-column chunk of wide conv layer i, split into
                PE segments so the scheduler can software-pipeline the
                in-order PE queue (the 1x1 waits ~2us for tanh/sig/mul;
                another unit's taps fill that hole)."""
                N = min(CH, T_out - c0)
                Nmm = N + (N & 1)
                cell = {}

                def seg0():
                    aps = ps.tile([C, Nmm], F32, tag="aps")
                    gps = ps.tile([C, Nmm], F32, tag="gps")
                    for half, pt in (("a", aps), ("g", gps)):
                        for j, (src, off) in enumerate(taps):
                            nc.tensor.matmul(
                                pt[:], wsl(half, i, j),
                                src[:, off + c0: off + c0 + Nmm],
                                start=(j == 0), stop=(j == 3))
                    ta = sb.tile([C, Nmm], BF16, tag="ta", bufs=4)
                    sg = sb.tile([C, Nmm], BF16, tag="sg", bufs=4)
                    nc.scalar.activation(ta[:], aps[:], AF.Tanh,
                                         bias=bap("a", i))
                    nc.scalar.activation(sg[:], gps[:], AF.Sigmoid,
                                         bias=bap("g", i))
                    x2 = sb.tile([128, Nmm], BF16, tag="x2", bufs=4)
                    nc.gpsimd.tensor_mul(x2[:C], ta[:], sg[:])
                    cell["x2"] = x2

                def seg1():
                    xps = ps.tile([C, Nmm], F32, tag="xps")
                    nc.tensor.matmul(xps[:], wsl("1", i), cell["x2"][:],
                                     start=True, stop=True)
                    return write_out(c0, N, xps[:, :N])

                return [seg0, seg1]

            def encoder_units(s, g, is_query):
                """Chunk-level emitters for the scheduler.

                Layer 3 fuses the f0/f1 head; VQ blocks (search) are
                separate units paced by the scheduler."""
                T0h, T1 = g["T0h"], g["T1"]
                E1, O1 = g["E1"], g["O1"]
                T2, T3, T4 = g["T2"], g["T3"], g["T4"]
                d_e, d_o = (d_qe, d_qo) if is_query else (d_se, d_so)
                sfx = f"{'q' if is_query else 's'}{s}"
                st = {}

                def alloc0():
                    x0e = sb.tile([128, T0h + 8], BF16, tag=f"x0e{sfx}")
                    x0o = sb.tile([128, T0h + 8], BF16, tag=f"x0o{sfx}")
                    nc.vector.memset(x0e[:, T0h:], 0.0)
                    nc.vector.memset(x0o[:, T0h:], 0.0)
                    st.update(x0e=x0e, x0o=x0o)

                def load_dma(c0, n):
                    n = min(n, T0h - c0)
                    nc.sync.dma_start(st["x0e"][:, c0:c0 + n],
                                      d_e[s, :, c0:c0 + n])
                    nc.sync.dma_start(st["x0o"][:, c0:c0 + n],
                                      d_o[s, :, c0:c0 + n])

                def alloc_a():
                    x1e = sb.tile([128, E1 + 8], BF16, tag=f"x1e{sfx}")
                    x1o = sb.tile([128, O1 + 8], BF16, tag=f"x1o{sfx}")
                    nc.vector.memset(x1e[:, E1:], 0.0)
                    nc.vector.memset(x1o[:, O1:], 0.0)
                    nc.sync.dma_start(x1e[C:, :E1], d_z[:, :E1])
                    nc.sync.dma_start(x1o[C:, :O1], d_z[:, :O1])
                    st.update(x1e=x1e, x1o=x1o)

                def alloc_b():
                    x2f = sb.tile([128, T2 + 8], BF16, tag=f"x2f{sfx}")
                    nc.vector.memset(x2f[:, T2:], 0.0)
                    nc.sync.dma_start(x2f[C:, :T2], d_z[:, :T2])
                    x3f = sb.tile([128, T3 + 8], BF16, tag=f"x3f{sfx}")
                    nc.vector.memset(x3f[:, T3:], 0.0)
                    nc.sync.dma_start(x3f[C:, :T3], d_z[:, :T3])
                    st.update(x2f=x2f, x3f=x3f)

                def alloc_c():
                    x4f = sb.tile([128, T4 + 8], BF16, tag=f"x4f{sfx}")
                    nc.vector.memset(x4f[:, T4:], 0.0)
                    nc.sync.dma_start(x4f[C:, :T4], d_z[:, :T4])
                    # f1(relu(f0(x))); rows 80/81 of enc are const 1.0
                    # (zero weights, bias 1) for the VQ score offsets.
                    enc = sb.tile([128, T4], BF16, tag=f"enc{sfx}")
                    nc.sync.dma_start(enc[M_F1:, :], d_z[:128 - M_F1, :T4])
                    st.update(x4f=x4f, enc=enc)

                def w0(c0, N, xps):
                    ne, no = (N + 1) // 2, N // 2
                    h = c0 // 2
                    nc.scalar.activation(
                        st["x1e"][:C, h:h + ne], xps[:, 0:N:2], AF.Copy)
                    nc.vector.tensor_copy(
                        st["x1o"][:C, h:h + no], xps[:, 1:N:2])

                def w1(c0, N, xps):
                    nc.vector.tensor_add(
                        st["x2f"][:C, c0:c0 + N], xps,
                        st["x1o"][:C, c0 + 1:c0 + 1 + N])

                def w2(c0, N, xps):
                    nc.vector.tensor_add(
                        st["x3f"][:C, c0:c0 + N], xps,
                        st["x2f"][:C, c0 + 3:c0 + 3 + N])

                def w3(c0, N, xps):
                    x4f, x3f, enc = st["x4f"], st["x3f"], st["enc"]
                    nc.vector.tensor_add(
                        x4f[:C, c0:c0 + N], xps,
                        x3f[:C, c0 + 3:c0 + 3 + N])
                    cf = {}

                    def seg_f0():
                        p0 = ps.tile([C, N], F32, tag="xps")
                        nc.tensor.matmul(p0[:], wsl("f0", 0),
                                         x4f[:, c0:c0 + N],
                                         start=True, stop=True)
                        xf = sb.tile([128, N], BF16, tag="xf", bufs=4)
                        nc.scalar.activation(xf[:C], p0[:], AF.Relu,
                                             bias=bap("f0"))
                        cf["xf"] = xf

                    def seg_f1():
                        p1 = ps.tile([M_F1, N], F32, tag="xps")
                        nc.tensor.matmul(p1[:], wpk[:, 2960:2960 + M_F1],
                                         cf["xf"][:], start=True, stop=True)
                        nc.scalar.activation(enc[:M_F1, c0:c0 + N], p1[:],
                                             AF.Identity, bias=bap("f1"))

                    return [seg_f0, seg_f1]

                Ts = [T1, T2, T3, T4]
                writers = [w0, w1, w2, w3]

                def taps(i):
                    if i == 0:
                        return [(st["x0e"], 0), (st["x0o"], 0),
                                (st["x0e"], 1), (st["x0o"], 1)]
                    if i == 1:
                        return [(st["x1e"], 0), (st["x1o"], 0),
                                (st["x1e"], 1), (st["x1o"], 1)]
                    x = st["x2f"] if i == 2 else st["x3f"]
                    return [(x, 0), (x, 1), (x, 2), (x, 3)]

                def emit_segs(i, c):
                    return wide_chunk_segs(i, taps(i), Ts[i], writers[i],
                                           c * CH)

                nch = [-(-t // CH) for t in Ts]
                ins = [E1, T2, T3]

                def req(i, c):
                    """Chunks of layer i-1 needed before chunk c of layer i."""
                    per = CH // 2 if i == 1 else CH
                    need = min(CH * c + 515, ins[i - 1])
                    return min(nch[i - 1], -(-need // per))

                return dict(alloc0=alloc0, load_dma=load_dma, alloc_a=alloc_a,
                            alloc_b=alloc_b, alloc_c=alloc_c,
                            emit_segs=emit_segs, nch=nch, req=req, st=st)

            def run_sched(encs, prios, vqs=None, post_l3=None):
                """Software-pipelined emission: each conv unit is a list
                of PE segments (taps / 1x1 / f0 / f1); a segment is only
                emitted one turn after its predecessor so another unit's
                taps fill the PE queue between dependent matmuls.  VQ
                (matmul, max-reduce) singles are paced in between."""
                sprog = [[0] * 4 for _ in encs]   # units started
                wprog = [[0] * 4 for _ in encs]   # units fully emitted
                conv_rem = sum(sum(e["nch"]) for e in encs)
                segq = []                          # [closures, sidx, i, turn]
                vprog = {sidx: 0 for sidx in (vqs or {})}
                vtot = {sidx: 3 * v["nblk"] for sidx, v in (vqs or {}).items()}
                turn = 0

                def unit_done(sidx, i):
                    wprog[sidx][i] += 1
                    if (i == 3 and wprog[sidx][3] == encs[sidx]["nch"][3]
                            and post_l3):
                        post_l3(sidx)

                def run_seg(entry):
                    closures, sidx, i, _ = entry
                    more = closures.pop(0)()
                    if more:
                        closures.extend(more)
                    if closures:
                        segq.append([closures, sidx, i, turn])
                    else:
                        unit_done(sidx, i)

                def try_new(group):
                    nonlocal conv_rem
                    for k in range(len(group)):
                        sidx = group[(turn + k) % len(group)]
                        e, p = encs[sidx], sprog[sidx]
                        for i in (3, 2, 1, 0):
                            if p[i] >= e["nch"][i]:
                                continue
                            if i > 0 and wprog[sidx][i - 1] < e["req"](i, p[i]):
                                continue
                            segs = e["emit_segs"](i, p[i])
                            p[i] += 1
                            conv_rem -= 1
                            run_seg([segs, sidx, i, turn])
                            return True
                    return False

                def emit_vq(budget):
                    done = 0
                    keys = sorted(vqs)
                    for k in range(len(keys)):
                        if done >= budget:
                            break
                        sidx = keys[(turn + k) % len(keys)]
                        v = vqs[sidx]
                        while vprog[sidx] < vtot[sidx] and done < budget:
                            b, ti = divmod(vprog[sidx], 3)
                            if wprog[sidx][3] < v["req"](b):
                                break
                            v["emit_one"](b, ti)
                            vprog[sidx] += 1
                            done += 1
                    return done

                while (conv_rem or segq or
                       any(vprog[s] < vtot[s] for s in vprog)):
                    emitted = 0
                    # cap in-flight multi-segment units so the ring
                    # buffers (depth 4) can never cycle-deadlock
                    if len(segq) < 3:
                        for group in prios:
                            if try_new(group):
                                emitted += 1
                                break
                    if segq and segq[0][3] < turn:
                        run_seg(segq.pop(0))
                        emitted += 1
                    if vqs:
                        emitted += emit_vq(2 if emitted else 4)
                    if not emitted and segq:
                        run_seg(segq.pop(0))
                        emitted = 1
                    turn += 1
                    assert emitted, "scheduler deadlock"

            # ---- encoders: query inputs first; search streams behind and
            # runs in the middle; query sample 1 is held back to overlap
            # the VQ drain at the end ----
            qencs = [encoder_units(s, GEO_QUERY, True) for s in range(SPC)]
            sencs = [encoder_units(s, GEO_SEARCH, False) for s in range(SPC)]
            for e in qencs + sencs:
                e["alloc0"]()
            for c0 in (0, 512):
                for e in qencs:
                    e["load_dma"](c0, 512)
            for e in qencs:
                e["alloc_a"]()
                e["alloc_b"]()
                e["alloc_c"]()
            # remaining static tables arrive behind the query inputs
            nc.sync.dma_start(wpk[:, LBLK:], d_wpk[:, LBLK:])
            epk3 = sb.tile([128, 3 * NK], BF16, tag="epk3")
            nc.sync.dma_start(epk3[:], d_epk3[:])
            wlt = sb.tile([128, 2], BF16, tag="wlt")
            nc.sync.dma_start(wlt[:], d_wlt[:])
            brow = sb.tile([1, 4], F32, tag="brow")
            nc.sync.dma_start(brow[:], d_blt[:])
            ztail = sb.tile([2, 32], F32, tag="ztail")
            nc.sync.dma_start(ztail[:], d_ztail[:])
            # search inputs interleaved with their zero-fills, ordered so
            # each layer's zeros land before that layer can be scheduled
            for e in sencs:
                e["load_dma"](0, 1024)
            for e in sencs:
                e["alloc_a"]()
            for e in sencs:
                e["load_dma"](1024, 1024)
            for e in sencs:
                e["alloc_b"]()
            for e in sencs:
                e["load_dma"](2048, 1024)
            for e in sencs:
                e["alloc_c"]()
            for e in sencs:
                e["load_dma"](3072, 1024)

            T4s = GEO_SEARCH["T4"]
            accs = {}
            for s in range(SPC):
                mt = sb.tile([128, 16], F32, tag=f"mt{s}")
                u0t = sb.tile([128, 16], F32, tag=f"u0t{s}")
                u1t = sb.tile([128, 16], F32, tag=f"u1t{s}")
                nc.vector.memset(mt[:], NEG)
                nc.vector.memset(u0t[:], NEG)
                nc.vector.memset(u1t[:], NEG)
                accs[s] = (mt, u0t, u1t)

            import concourse.bass_isa as bass_isa
            # warm the Q7 IRAM for partition_all_reduce early so the
            # final reduce doesn't pay the ~6us ext-isa load at the tail
            zwarm = sb.tile([128, 16], F32, tag="zwarm")
            nc.gpsimd.partition_all_reduce(zwarm[:], accs[0][0][:],
                                           channels=128,
                                           reduce_op=bass_isa.ReduceOp.max)

            zred = sb.tile([128, 4], F32, tag="zred")
            vts = {}

            def emit_vpath(s):
                """v = w_lin @ enc_q, tiled x4 through DRAM into [128,16]."""
                enc_q = qencs[s]["st"]["enc"]
                vps = ps.tile([128, NK], F32, tag="vq")
                nc.tensor.matmul(vps[:2, :504], wlt[:], enc_q[:, :504],
                                 start=True, stop=True)
                vrow = sb.tile([2, 504], F32, tag="vrow", bufs=2)
                nc.scalar.activation(vrow[:], vps[:2, :504], AF.Copy)
                for k in range(4):
                    nc.sync.dma_start(
                        d_zbuf[2 * s:2 * s + 2, 504 * k:504 * (k + 1)],
                        vrow[:])
                nc.sync.dma_start(d_zbuf[2 * s:2 * s + 2, 2016:], ztail[:])
                vt0 = sb.tile([128, 16], F32, tag=f"vt{2 * s}")
                vt1 = sb.tile([128, 16], F32, tag=f"vt{2 * s + 1}")
                nc.sync.dma_start(
                    vt0[:], d_zbuf[2 * s].rearrange("(b p) -> p b", p=128))
                nc.sync.dma_start(
                    vt1[:], d_zbuf[2 * s + 1].rearrange("(b p) -> p b", p=128))
                vts[s] = (vt0, vt1)

            def make_vq(s):
                tgts = accs[s]
                st = sencs[s]["st"]

                def emit_one(b, ti):
                    t0 = 128 * b
                    P = min(128, T4s - t0)
                    sps = ps.tile([128, NK], F32, tag="vq")
                    nc.tensor.matmul(
                        sps[:P, :], st["enc"][:, t0:t0 + P],
                        epk3[:, NK * ti:NK * (ti + 1)],
                        start=True, stop=True)
                    nc.vector.tensor_reduce(
                        tgts[ti][:P, b:b + 1], sps[:P, :], axis=AX.X,
                        op=OP.max)

                def req(b):
                    return min(sencs[s]["nch"][3], -(-(128 * (b + 1)) // CH))

                return dict(emit_one=emit_one, req=req, nblk=16)

            def post_l3(sidx):
                if sidx < SPC:          # query sample finished its encoder
                    emit_vpath(sidx)

            vqs = {SPC + s: make_vq(s) for s in range(SPC)}
            run_sched(qencs + sencs,
                      prios=[[0], [2, 3], [1]], vqs=vqs, post_l3=post_l3)

            # ---- z = (u - m)*BIG + vt, reduce ----
            for s in range(SPC):
                mt, u0t, u1t = accs[s]
                for j, ut in ((0, u0t), (1, u1t)):
                    zt = sb.tile([128, 16], F32, tag="zt", bufs=2)
                    nc.vector.tensor_sub(zt[:], ut[:], mt[:])
                    nc.vector.scalar_tensor_tensor(
                        out=zt[:], in0=zt[:], scalar=BIG, in1=vts[s][j][:],
                        op0=OP.mult, op1=OP.add)
                    nc.vector.tensor_reduce(
                        zred[:, 2 * s + j:2 * s + j + 1], zt[:], axis=AX.X,
                        op=OP.max)

            zar = sb.tile([128, 4], F32, tag="zar")
            nc.gpsimd.partition_all_reduce(zar[:], zred[:], channels=128,
                                           reduce_op=bass_isa.ReduceOp.max)
            zrow = sb.tile([1, 4], F32, tag="zrow")
            nc.vector.tensor_add(zrow[:], zar[0:1, :], brow[:])
            outv = sb.tile([1, 4], F32, tag="outv")
            nc.scalar.activation(outv[:], zrow[:], AF.Tanh)
            nc.sync.dma_start(d_out[:], outv[:])

    nc.finalize()
    return nc


_NC_CACHE = None


def _get_nc():
    global _NC_CACHE
    if _NC_CACHE is None:
        _NC_CACHE = _build()
    return _NC_CACHE


def prep_inputs(search, query, w_wide, b_wide, w_1x1, b_1x1, w_f0, b_f0,
                w_f1, b_f1, embedding, w_lin, b_lin):
    """Host-side packing -> list of per-core input maps (bf16 operands)."""
    f = np.float32
    bf = ml_dtypes.bfloat16
    search = np.asarray(search, f)
    query = np.asarray(query, f)

    def pad128(a):
        # (N, T, C) -> (N, 128, T) channel-major, rows C..127 zero
        n, t, _ = a.shape
        out = np.zeros((n, 128, t), dtype=bf)
        out[:, :C, :] = a.transpose(0, 2, 1).astype(bf)
        return out

    se = pad128(search[:, 0::2, :])
    so = pad128(search[:, 1::2, :])
    qe = pad128(query[:, 0::2, :])
    qo = pad128(query[:, 1::2, :])

    w_wide = np.asarray(w_wide, f)
    w_1x1 = np.asarray(w_1x1, f)
    b_wide = np.asarray(b_wide, f)
    b_1x1 = np.asarray(b_1x1, f)
    cols = []
    for i in range(4):
        for j in range(4):
            cols.append(w_wide[i, :C, :, j].T)     # a taps
        for j in range(4):
            cols.append(w_wide[i, C:, :, j].T)     # g taps
        cols.append(w_1x1[i, :, :, 0].T)
    cols.append(np.asarray(w_f0, f)[:, :, 0].T)
    wf1 = np.zeros((C, M_F1), f)
    wf1[:, :C] = np.asarray(w_f1, f)[:, :, 0].T   # cols 80/81 stay zero
    cols.append(wf1)
    wpk80 = np.ascontiguousarray(np.concatenate(cols, axis=1))
    assert wpk80.shape == (C, WPACK_COLS)
    wpk = np.zeros((128, WPACK_COLS), dtype=bf)
    wpk[:C] = wpk80.astype(bf)

    # cumulative 1x1 biases folded into next layer's tap biases / f0 bias
    cum = np.cumsum(b_1x1, axis=0)                 # cum[i] = b0+..+bi
    bcols = []
    for i in range(4):                             # a-tap biases
        ba = b_wide[i, :C].copy()
        if i > 0:
            ba += w_wide[i, :C].sum(axis=2) @ cum[i - 1]
        bcols.append(ba)
    for i in range(4):                             # g-tap biases
        bg = b_wide[i, C:].copy()
        if i > 0:
            bg += w_wide[i, C:].sum(axis=2) @ cum[i - 1]
        bcols.append(bg)
    bcols += [np.zeros(C, f)] * 4                  # (1x1 biases folded away)
    bf0 = np.asarray(b_f0, f) + np.asarray(w_f0, f)[:, :, 0] @ cum[3]
    bcols += [bf0, np.asarray(b_f1, f)]
    bpk = np.zeros((M_F1, 14), f)
    bpk[:C] = np.stack(bcols, axis=1)
    bpk[C, _b_off("f1")] = 1.0     # f1 rows 80/81 = 0*x + 1.0 -> const-1
    bpk[C + 1, _b_off("f1")] = 1.0

    emb = np.asarray(embedding, f)[0]            # (512, 80)
    e2 = (emb.astype(np.float64) ** 2).sum(1)
    ew = (emb.astype(np.float64) @ np.asarray(w_lin, f).T.astype(np.float64))
    epk3 = np.zeros((128, 3 * NK), f)
    for ti in range(3):
        epk3[:C, NK * ti:NK * (ti + 1)] = emb.T
        epk3[C, NK * ti:NK * (ti + 1)] = -0.5 * e2
    epk3[C + 1, NK:2 * NK] = ew[:, 0] / BIG
    epk3[C + 1, 2 * NK:3 * NK] = ew[:, 1] / BIG
    epk3 = epk3.astype(bf)
    z48 = np.zeros((48, 4104), dtype=bf)
    ztail = np.zeros((2, 32), f)
    ztail[:, 24:] = NEG
    wlt = np.zeros((128, 2), dtype=bf)
    wlt[:C] = np.ascontiguousarray(np.asarray(w_lin, f).T).astype(bf)
    b_lin = np.asarray(b_lin, f)
    blt = np.array([[b_lin[0], b_lin[1], b_lin[0], b_lin[1]]], f)

    maps = []
    for c in range(NCORES):
        sl = slice(SPC * c, SPC * (c + 1))
        maps.append({
            "se": se[sl], "so": so[sl], "qe": qe[sl], "qo": qo[sl],
            "wpk": wpk, "bpk": bpk, "epk3": epk3, "wlt": wlt, "blt": blt,
            "z48": z48, "ztail": ztail,
        })
    return maps


def kernel(**inputs):
    nc = _get_nc()
    maps = prep_inputs(**inputs)
    res = run_bass_kernel_spmd(nc, maps, core_ids=list(range(NCORES)))
    out = np.concatenate([r["out"].reshape(SPC, 2) for r in res.results],
                         axis=0)
    return out.astype(np.float32)


if __name__ == "__main__":
    import reference
    inputs = {k: np.asarray(v) for k, v in reference.setup_inputs().items()}
    got = kernel(**inputs)
    print(got)


# revision 23
# speedup vs baseline: 1.1248x; 1.1248x over previous
"""AudioFinder Trainium2 kernel.

Data parallel over batch: 16 samples -> 8 cores x 2 samples.

Per-core pipeline (bf16 matmuls / f32 psum, both samples interleaved
layer-by-layer so one sample's matmuls fill the other's pipeline-latency
bubbles on the in-order engine queues):
  1. Both query encoders (T=2048 -> 504), layers interleaved; v =
     w_lin @ enc_q; the tiled-x4 + pad row [1,2048] is bounced through
     DRAM into [128,16] (t = p + 128*b) while the search encoders run.
  2. Both search encoders (T=8192 -> 2040), layers interleaved, with
     the f0/f1 head fused into layer 3 and VQ 128-t blocks emitted as
     separate scheduler units paced between conv chunks so the DVE
     max-reduces never back up the PE's PSUM banks.
  3. VQ per 128-t block: three bf16 matmuls (enc block stationary)
     against epk3's three NK-column blocks, DVE max-reduce each:
     scores s[t,k] = enc_s[t]@emb[k] - |emb[k]|^2/2 (enc rows 80/81
     const 1.0; epk3 row 80 carries -|e|^2/2 shared, row 81 carries
     0 / ew0/BIG / ew1/BIG with ew = emb @ w_lin.T):
       u_j[t] = max_k (s[t,k] + ew[k,j]/BIG),  m[t] = max_k s[t,k]
       => (u_j - m)*BIG = ew[argmax_k s, j]   (fp32-psum exact)
  4. z = (u-m)*BIG + vt in [128,16]; max over free dim on DVE, across
     partitions on Pool; out = tanh(max z + b_lin).

All 1x1-conv biases are folded (host-side) into the next layer's tap
biases / f0's bias, so the residual writers are plain psum+residual
adds on DVE and layer-0's writer is a pair of ACT copies.  Every
matmul keeps a 128-partition stationary operand (zero rows beyond 80)
so FWL stays enabled; x2/xf ring rows 80-127 are zeroed once at start.
Gated tanh*sigmoid on ACT, gate product on Pool.
"""

import numpy as np
import ml_dtypes

import concourse.bacc as bacc
import concourse.mybir as mybir
import concourse.tile as tile
from concourse.bass_utils import run_bass_kernel_spmd

F32 = mybir.dt.float32
BF16 = mybir.dt.bfloat16
AF = mybir.ActivationFunctionType
OP = mybir.AluOpType
AX = mybir.AxisListType

NCORES = 8
SPC = 2          # samples per core
C = 80
NK = 512         # codebook size
BIG = 1024.0
NEG = -1e30
CH = 512         # chunk (free-dim) size

# layer geometry
GEO_SEARCH = dict(T0h=4096, T1=4095, E1=2048, O1=2047, T2=2046, T3=2043, T4=2040)
GEO_QUERY = dict(T0h=1024, T1=1023, E1=512, O1=511, T2=510, T3=507, T4=504)

# wpack layout: layer-major [a_i(4 taps), g_i(4 taps), w1x1_i] x 4, f0, f1
LBLK = 720  # 4*80 + 4*80 + 80

def _w_off(kind, i, j=0):
    if kind == "a":
        return LBLK * i + C * j
    if kind == "g":
        return LBLK * i + 320 + C * j
    if kind == "1":
        return LBLK * i + 640
    if kind == "f0":
        return 2880
    if kind == "f1":
        return 2960
    raise KeyError(kind)


M_F1 = 82  # f1 conv emits 80 real channels + two const-1 channels


WPACK_COLS = 3042
# bias pack columns: ba0..3, bg0..3, b10..3, bf0, bf1
def _b_off(kind, i=0):
    return {"a": i, "g": 4 + i, "1": 8 + i, "f0": 12, "f1": 13}[kind]


def _build():
    nc = bacc.Bacc("TRN2", target_bir_lowering=False, debug=False,
                   num_devices=NCORES)
    d_se = nc.dram_tensor("se", [SPC, 128, 4096], BF16, kind="ExternalInput")
    d_so = nc.dram_tensor("so", [SPC, 128, 4096], BF16, kind="ExternalInput")
    d_qe = nc.dram_tensor("qe", [SPC, 128, 1024], BF16, kind="ExternalInput")
    d_qo = nc.dram_tensor("qo", [SPC, 128, 1024], BF16, kind="ExternalInput")
    d_wpk = nc.dram_tensor("wpk", [128, WPACK_COLS], BF16, kind="ExternalInput")
    d_bpk = nc.dram_tensor("bpk", [M_F1, 14], F32, kind="ExternalInput")
    d_epk3 = nc.dram_tensor("epk3", [128, 3 * NK], BF16, kind="ExternalInput")
    d_z = nc.dram_tensor("z48", [48, 4104], BF16, kind="ExternalInput")
    d_ztail = nc.dram_tensor("ztail", [2, 32], F32, kind="ExternalInput")
    d_wlt = nc.dram_tensor("wlt", [128, 2], BF16, kind="ExternalInput")
    d_blt = nc.dram_tensor("blt", [1, 4], F32, kind="ExternalInput")
    d_out = nc.dram_tensor("out", [1, 4], F32, kind="ExternalOutput")
    d_zbuf = nc.dram_tensor("zbuf", [2 * SPC, 2048], F32)

    with tile.TileContext(nc) as tc:
        with (
            tc.tile_pool(name="sb", bufs=1) as sb,
            tc.tile_pool(name="ps", bufs=2, space="PSUM") as ps,
        ):
            # ---- startup DMAs, ordered so layer-0 work can start early ----
            wpk = sb.tile([128, WPACK_COLS], BF16, tag="wpk")
            nc.sync.dma_start(wpk[:, :LBLK], d_wpk[:, :LBLK])
            bpk = sb.tile([M_F1, 14], F32, tag="bpk")
            nc.sync.dma_start(bpk[:], d_bpk[:])
            # zero rows 80-127 of the x2/xf ring buffers once, so the
            # 1x1/f1 matmuls can use 128-partition operands (FWL on)
            for _ in range(4):
                x2t = sb.tile([128, CH], BF16, tag="x2", bufs=4)
                nc.vector.memset(x2t[64:, :], 0.0)
                xft = sb.tile([128, CH], BF16, tag="xf", bufs=4)
                nc.vector.memset(xft[64:, :], 0.0)

            def wsl(kind, i, j=0, rows=128):
                off = _w_off(kind, i, j)
                return wpk[:rows, off:off + C]

            def bap(kind, i=0):
                o = _b_off(kind, i)
                n = M_F1 if kind == "f1" else C
                return bpk[:n, o:o + 1]

            def wide_chunk_segs(i, taps, T_out, write_out, c0):
                """One CH-column chunk of wide conv layer i, split into
                PE segments so the scheduler can software-pipeline the
                in-order PE queue (the 1x1 waits ~2us for tanh/sig/mul;
                another unit's taps fill that hole)."""
                N = min(CH, T_out - c0)
                Nmm = N + (N & 1)
                cell = {}

                def seg0():
                    aps = ps.tile([C, Nmm], F32, tag="aps")
                    gps = ps.tile([C, Nmm], F32, tag="gps")
                    for half, pt in (("a", aps), ("g", gps)):
                        for j, (src, off) in enumerate(taps):
                            nc.tensor.matmul(
                                pt[:], wsl(half, i, j),
                                src[:, off + c0: off + c0 + Nmm],
                                start=(j == 0), stop=(j == 3))
                    ta = sb.tile([C, Nmm], BF16, tag="ta", bufs=4)
                    sg = sb.tile([C, Nmm], BF16, tag="sg", bufs=4)
                    nc.scalar.activation(ta[:], aps[:], AF.Tanh,
                                         bias=bap("a", i))
                    nc.scalar.activation(sg[:], gps[:], AF.Sigmoid,
                                         bias=bap("g", i))
                    x2 = sb.tile([128, Nmm], BF16, tag="x2", bufs=4)
                    nc.gpsimd.tensor_mul(x2[:C], ta[:], sg[:])
                    cell["x2"] = x2

                def seg1():
                    xps = ps.tile([C, Nmm], F32, tag="xps")
                    nc.tensor.matmul(xps[:], wsl("1", i), cell["x2"][:],
                                     start=True, stop=True)
                    return write_out(c0, N, xps[:, :N])

                return [seg0, seg1]

            def encoder_units(s, g, is_query):
                """Chunk-level emitters for the scheduler.

                Layer 3 fuses the f0/f1 head; VQ blocks (search) are
                separate units paced by the scheduler."""
                T0h, T1 = g["T0h"], g["T1"]
                E1, O1 = g["E1"], g["O1"]
                T2, T3, T4 = g["T2"], g["T3"], g["T4"]
                d_e, d_o = (d_qe, d_qo) if is_query else (d_se, d_so)
                sfx = f"{'q' if is_query else 's'}{s}"
                st = {}

                def alloc0():
                    x0e = sb.tile([128, T0h + 8], BF16, tag=f"x0e{sfx}")
                    x0o = sb.tile([128, T0h + 8], BF16, tag=f"x0o{sfx}")
                    nc.vector.memset(x0e[:, T0h:], 0.0)
                    nc.vector.memset(x0o[:, T0h:], 0.0)
                    st.update(x0e=x0e, x0o=x0o)

                def load_dma(c0, n):
                    n = min(n, T0h - c0)
                    nc.sync.dma_start(st["x0e"][:, c0:c0 + n],
                                      d_e[s, :, c0:c0 + n])
                    nc.sync.dma_start(st["x0o"][:, c0:c0 + n],
                                      d_o[s, :, c0:c0 + n])

                def alloc_a():
                    x1e = sb.tile([128, E1 + 8], BF16, tag=f"x1e{sfx}")
                    x1o = sb.tile([128, O1 + 8], BF16, tag=f"x1o{sfx}")
                    nc.vector.memset(x1e[:, E1:], 0.0)
                    nc.vector.memset(x1o[:, O1:], 0.0)
                    nc.sync.dma_start(x1e[C:, :E1], d_z[:, :E1])
                    nc.sync.dma_start(x1o[C:, :O1], d_z[:, :O1])
                    st.update(x1e=x1e, x1o=x1o)

                def alloc_b():
                    x2f = sb.tile([128, T2 + 8], BF16, tag=f"x2f{sfx}")
                    nc.vector.memset(x2f[:, T2:], 0.0)
                    nc.sync.dma_start(x2f[C:, :T2], d_z[:, :T2])
                    x3f = sb.tile([128, T3 + 8], BF16, tag=f"x3f{sfx}")
                    nc.vector.memset(x3f[:, T3:], 0.0)
                    nc.sync.dma_start(x3f[C:, :T3], d_z[:, :T3])
                    st.update(x2f=x2f, x3f=x3f)

                def alloc_c():
                    x4f = sb.tile([128, T4 + 8], BF16, tag=f"x4f{sfx}")
                    nc.vector.memset(x4f[:, T4:], 0.0)
                    nc.sync.dma_start(x4f[C:, :T4], d_z[:, :T4])
                    # f1(relu(f0(x))); rows 80/81 of enc are const 1.0
                    # (zero weights, bias 1) for the VQ score offsets.
                    enc = sb.tile([128, T4], BF16, tag=f"enc{sfx}")
                    nc.sync.dma_start(enc[M_F1:, :], d_z[:128 - M_F1, :T4])
                    st.update(x4f=x4f, enc=enc)

                def w0(c0, N, xps):
                    ne, no = (N + 1) // 2, N // 2
                    h = c0 // 2
                    nc.scalar.activation(
                        st["x1e"][:C, h:h + ne], xps[:, 0:N:2], AF.Copy)
                    nc.vector.tensor_copy(
                        st["x1o"][:C, h:h + no], xps[:, 1:N:2])

                def w1(c0, N, xps):
                    nc.vector.tensor_add(
                        st["x2f"][:C, c0:c0 + N], xps,
                        st["x1o"][:C, c0 + 1:c0 + 1 + N])

                def w2(c0, N, xps):
                    nc.vector.tensor_add(
                        st["x3f"][:C, c0:c0 + N], xps,
                        st["x2f"][:C, c0 + 3:c0 + 3 + N])

                def w3(c0, N, xps):
                    x4f, x3f, enc = st["x4f"], st["x3f"], st["enc"]
                    nc.vector.tensor_add(
                        x4f[:C, c0:c0 + N], xps,
                        x3f[:C, c0 + 3:c0 + 3 + N])
                    cf = {}

                    def seg_f0():
                        p0 = ps.tile([C, N], F32, tag="xps")
                        nc.tensor.matmul(p0[:], wsl("f0", 0),
                                         x4f[:, c0:c0 + N],
                                         start=True, stop=True)
                        xf = sb.tile([128, N], BF16, tag="xf", bufs=4)
                        nc.scalar.activation(xf[:C], p0[:], AF.Relu,
                                             bias=bap("f0"))
                        cf["xf"] = xf

                    def seg_f1():
                        p1 = ps.tile([M_F1, N], F32, tag="xps")
                        nc.tensor.matmul(p1[:], wpk[:, 2960:2960 + M_F1],
                                         cf["xf"][:], start=True, stop=True)
                        nc.scalar.activation(enc[:M_F1, c0:c0 + N], p1[:],
                                             AF.Identity, bias=bap("f1"))

                    return [seg_f0, seg_f1]

                Ts = [T1, T2, T3, T4]
                writers = [w0, w1, w2, w3]

                def taps(i):
                    if i == 0:
                        return [(st["x0e"], 0), (st["x0o"], 0),
                                (st["x0e"], 1), (st["x0o"], 1)]
                    if i == 1:
                        return [(st["x1e"], 0), (st["x1o"], 0),
                                (st["x1e"], 1), (st["x1o"], 1)]
                    x = st["x2f"] if i == 2 else st["x3f"]
                    return [(x, 0), (x, 1), (x, 2), (x, 3)]

                def emit_segs(i, c):
                    return wide_chunk_segs(i, taps(i), Ts[i], writers[i],
                                           c * CH)

                nch = [-(-t // CH) for t in Ts]
                ins = [E1, T2, T3]

                def req(i, c):
                    """Chunks of layer i-1 needed before chunk c of layer i."""
                    per = CH // 2 if i == 1 else CH
                    need = min(CH * c + 515, ins[i - 1])
                    return min(nch[i - 1], -(-need // per))

                return dict(alloc0=alloc0, load_dma=load_dma, alloc_a=alloc_a,
                            alloc_b=alloc_b, alloc_c=alloc_c,
                            emit_segs=emit_segs, nch=nch, req=req, st=st)

            def run_sched(encs, prios, vqs=None, post_l3=None):
                """Software-pipelined emission: each conv unit is a list
                of PE segments (taps / 1x1 / f0 / f1); a segment is only
                emitted one turn after its predecessor so another unit's
                taps fill the PE queue between dependent matmuls.  VQ
                (matmul, max-reduce) singles are paced in between."""
                sprog = [[0] * 4 for _ in encs]   # units started
                wprog = [[0] * 4 for _ in encs]   # units fully emitted
                conv_rem = sum(sum(e["nch"]) for e in encs)
                segq = []                          # [closures, sidx, i, turn]
                vprog = {sidx: 0 for sidx in (vqs or {})}
                vtot = {sidx: 3 * v["nblk"] for sidx, v in (vqs or {}).items()}
                turn = 0

                def unit_done(sidx, i):
                    wprog[sidx][i] += 1
                    if (i == 3 and wprog[sidx][3] == encs[sidx]["nch"][3]
                            and post_l3):
                        post_l3(sidx)

                def run_seg(entry):
                    closures, sidx, i, _ = entry
                    more = closures.pop(0)()
                    if more:
                        closures.extend(more)
                    if closures:
                        segq.append([closures, sidx, i, turn])
                    else:
                        unit_done(sidx, i)

                def try_new(group):
                    nonlocal conv_rem
                    for k in range(len(group)):
                        sidx = group[(turn + k) % len(group)]
                        e, p = encs[sidx], sprog[sidx]
                        for i in (3, 2, 1, 0):
                            if p[i] >= e["nch"][i]:
                                continue
                            if i > 0 and wprog[sidx][i - 1] < e["req"](i, p[i]):
                                continue
                            segs = e["emit_segs"](i, p[i])
                            p[i] += 1
                            conv_rem -= 1
                            run_seg([segs, sidx, i, turn])
                            return True
                    return False

                def emit_vq(budget):
                    done = 0
                    keys = sorted(vqs)
                    for k in range(len(keys)):
                        if done >= budget:
                            break
                        sidx = keys[(turn + k) % len(keys)]
                        v = vqs[sidx]
                        while vprog[sidx] < vtot[sidx] and done < budget:
                            b, ti = divmod(vprog[sidx], 3)
                            if wprog[sidx][3] < v["req"](b):
                                break
                            v["emit_one"](b, ti)
                            vprog[sidx] += 1
                            done += 1
                    return done

                while (conv_rem or segq or
                       any(vprog[s] < vtot[s] for s in vprog)):
                    emitted = 0
                    # cap in-flight multi-segment units so the ring
                    # buffers (depth 4) can never cycle-deadlock
                    if len(segq) < 3:
                        for group in prios:
                            if try_new(group):
                                emitted += 1
                                break
                    if segq and segq[0][3] < turn:
                        run_seg(segq.pop(0))
                        emitted += 1
                        # drain backlogged segments faster at the conv tail
                        if len(segq) >= 2 and segq[0][3] < turn:
                            run_seg(segq.pop(0))
                            emitted += 1
                    if vqs:
                        emitted += emit_vq(3 if emitted else 8)
                    if not emitted and segq:
                        run_seg(segq.pop(0))
                        emitted = 1
                    turn += 1
                    assert emitted, "scheduler deadlock"

            # ---- encoders: query inputs first; search streams behind and
            # runs in the middle; query sample 1 is held back to overlap
            # the VQ drain at the end ----
            qencs = [encoder_units(s, GEO_QUERY, True) for s in range(SPC)]
            sencs = [encoder_units(s, GEO_SEARCH, False) for s in range(SPC)]
            for e in qencs + sencs:
                e["alloc0"]()
            for c0 in (0, 512):
                for e in qencs:
                    e["load_dma"](c0, 512)
            for e in qencs:
                e["alloc_a"]()
                e["alloc_b"]()
                e["alloc_c"]()
            # remaining static tables arrive behind the query inputs
            nc.sync.dma_start(wpk[:, LBLK:], d_wpk[:, LBLK:])
            epk3 = sb.tile([128, 3 * NK], BF16, tag="epk3")
            nc.sync.dma_start(epk3[:], d_epk3[:])
            wlt = sb.tile([128, 2], BF16, tag="wlt")
            nc.sync.dma_start(wlt[:], d_wlt[:])
            brow = sb.tile([1, 4], F32, tag="brow")
            nc.sync.dma_start(brow[:], d_blt[:])
            ztail = sb.tile([2, 32], F32, tag="ztail")
            nc.sync.dma_start(ztail[:], d_ztail[:])
            # search inputs interleaved with their zero-fills, ordered so
            # each layer's zeros land before that layer can be scheduled
            for e in sencs:
                e["load_dma"](0, 1024)
            for e in sencs:
                e["alloc_a"]()
            for e in sencs:
                e["load_dma"](1024, 1024)
            for e in sencs:
                e["alloc_b"]()
            for e in sencs:
                e["load_dma"](2048, 1024)
            for e in sencs:
                e["alloc_c"]()
            for e in sencs:
                e["load_dma"](3072, 1024)

            T4s = GEO_SEARCH["T4"]
            accs = {}
            for s in range(SPC):
                mt = sb.tile([128, 16], F32, tag=f"mt{s}")
                u0t = sb.tile([128, 16], F32, tag=f"u0t{s}")
                u1t = sb.tile([128, 16], F32, tag=f"u1t{s}")
                nc.vector.memset(mt[:], NEG)
                nc.vector.memset(u0t[:], NEG)
                nc.vector.memset(u1t[:], NEG)
                accs[s] = (mt, u0t, u1t)

            import concourse.bass_isa as bass_isa
            # warm the Q7 IRAM for partition_all_reduce early so the
            # final reduce doesn't pay the ~6us ext-isa load at the tail
            zwarm = sb.tile([128, 16], F32, tag="zwarm")
            nc.gpsimd.partition_all_reduce(zwarm[:], accs[0][0][:],
                                           channels=128,
                                           reduce_op=bass_isa.ReduceOp.max)

            zred = sb.tile([128, 4], F32, tag="zred")
            vts = {}

            def emit_vpath(s):
                """v = w_lin @ enc_q, tiled x4 through DRAM into [128,16]."""
                enc_q = qencs[s]["st"]["enc"]
                vps = ps.tile([128, NK], F32, tag="vq")
                nc.tensor.matmul(vps[:2, :504], wlt[:], enc_q[:, :504],
                                 start=True, stop=True)
                vrow = sb.tile([2, 504], F32, tag="vrow", bufs=2)
                nc.scalar.activation(vrow[:], vps[:2, :504], AF.Copy)
                for k in range(4):
                    nc.sync.dma_start(
                        d_zbuf[2 * s:2 * s + 2, 504 * k:504 * (k + 1)],
                        vrow[:])
                nc.sync.dma_start(d_zbuf[2 * s:2 * s + 2, 2016:], ztail[:])
                vt0 = sb.tile([128, 16], F32, tag=f"vt{2 * s}")
                vt1 = sb.tile([128, 16], F32, tag=f"vt{2 * s + 1}")
                nc.sync.dma_start(
                    vt0[:], d_zbuf[2 * s].rearrange("(b p) -> p b", p=128))
                nc.sync.dma_start(
                    vt1[:], d_zbuf[2 * s + 1].rearrange("(b p) -> p b", p=128))
                vts[s] = (vt0, vt1)

            def make_vq(s):
                tgts = accs[s]
                st = sencs[s]["st"]

                def emit_one(b, ti):
                    t0 = 128 * b
                    P = min(128, T4s - t0)
                    sps = ps.tile([128, NK], F32, tag="vq")
                    nc.tensor.matmul(
                        sps[:P, :], st["enc"][:, t0:t0 + P],
                        epk3[:, NK * ti:NK * (ti + 1)],
                        start=True, stop=True)
                    nc.vector.tensor_reduce(
                        tgts[ti][:P, b:b + 1], sps[:P, :], axis=AX.X,
                        op=OP.max)

                def req(b):
                    return min(sencs[s]["nch"][3], -(-(128 * (b + 1)) // CH))

                return dict(emit_one=emit_one, req=req, nblk=16)

            def post_l3(sidx):
                if sidx < SPC:          # query sample finished its encoder
                    emit_vpath(sidx)

            vqs = {SPC + s: make_vq(s) for s in range(SPC)}
            run_sched(qencs + sencs,
                      prios=[[0], [2, 3], [1]], vqs=vqs, post_l3=post_l3)

            # ---- z = (u - m)*BIG + vt, reduce ----
            for s in range(SPC):
                mt, u0t, u1t = accs[s]
                for j, ut in ((0, u0t), (1, u1t)):
                    zt = sb.tile([128, 16], F32, tag="zt", bufs=2)
                    nc.vector.tensor_sub(zt[:], ut[:], mt[:])
                    nc.vector.scalar_tensor_tensor(
                        out=zt[:], in0=zt[:], scalar=BIG, in1=vts[s][j][:],
                        op0=OP.mult, op1=OP.add)
                    nc.vector.tensor_reduce(
                        zred[:, 2 * s + j:2 * s + j + 1], zt[:], axis=AX.X,
                        op=OP.max)

            zar = sb.tile([128, 4], F32, tag="zar")
            nc.gpsimd.partition_all_reduce(zar[:], zred[:], channels=128,
                                           reduce_op=bass_isa.ReduceOp.max)
            zrow = sb.tile([1, 4], F32, tag="zrow")
            nc.vector.tensor_add(zrow[:], zar[0:1, :], brow[:])
            outv = sb.tile([1, 4], F32, tag="outv")
            nc.scalar.activation(outv[:], zrow[:], AF.Tanh)
            nc.sync.dma_start(d_out[:], outv[:])

    nc.finalize()
    return nc


_NC_CACHE = None


def _get_nc():
    global _NC_CACHE
    if _NC_CACHE is None:
        _NC_CACHE = _build()
    return _NC_CACHE


def prep_inputs(search, query, w_wide, b_wide, w_1x1, b_1x1, w_f0, b_f0,
                w_f1, b_f1, embedding, w_lin, b_lin):
    """Host-side packing -> list of per-core input maps (bf16 operands)."""
    f = np.float32
    bf = ml_dtypes.bfloat16
    search = np.asarray(search, f)
    query = np.asarray(query, f)

    def pad128(a):
        # (N, T, C) -> (N, 128, T) channel-major, rows C..127 zero
        n, t, _ = a.shape
        out = np.zeros((n, 128, t), dtype=bf)
        out[:, :C, :] = a.transpose(0, 2, 1).astype(bf)
        return out

    se = pad128(search[:, 0::2, :])
    so = pad128(search[:, 1::2, :])
    qe = pad128(query[:, 0::2, :])
    qo = pad128(query[:, 1::2, :])

    w_wide = np.asarray(w_wide, f)
    w_1x1 = np.asarray(w_1x1, f)
    b_wide = np.asarray(b_wide, f)
    b_1x1 = np.asarray(b_1x1, f)
    cols = []
    for i in range(4):
        for j in range(4):
            cols.append(w_wide[i, :C, :, j].T)     # a taps
        for j in range(4):
            cols.append(w_wide[i, C:, :, j].T)     # g taps
        cols.append(w_1x1[i, :, :, 0].T)
    cols.append(np.asarray(w_f0, f)[:, :, 0].T)
    wf1 = np.zeros((C, M_F1), f)
    wf1[:, :C] = np.asarray(w_f1, f)[:, :, 0].T   # cols 80/81 stay zero
    cols.append(wf1)
    wpk80 = np.ascontiguousarray(np.concatenate(cols, axis=1))
    assert wpk80.shape == (C, WPACK_COLS)
    wpk = np.zeros((128, WPACK_COLS), dtype=bf)
    wpk[:C] = wpk80.astype(bf)

    # cumulative 1x1 biases folded into next layer's tap biases / f0 bias
    cum = np.cumsum(b_1x1, axis=0)                 # cum[i] = b0+..+bi
    bcols = []
    for i in range(4):                             # a-tap biases
        ba = b_wide[i, :C].copy()
        if i > 0:
            ba += w_wide[i, :C].sum(axis=2) @ cum[i - 1]
        bcols.append(ba)
    for i in range(4):                             # g-tap biases
        bg = b_wide[i, C:].copy()
        if i > 0:
            bg += w_wide[i, C:].sum(axis=2) @ cum[i - 1]
        bcols.append(bg)
    bcols += [np.zeros(C, f)] * 4                  # (1x1 biases folded away)
    bf0 = np.asarray(b_f0, f) + np.asarray(w_f0, f)[:, :, 0] @ cum[3]
    bcols += [bf0, np.asarray(b_f1, f)]
    bpk = np.zeros((M_F1, 14), f)
    bpk[:C] = np.stack(bcols, axis=1)
    bpk[C, _b_off("f1")] = 1.0     # f1 rows 80/81 = 0*x + 1.0 -> const-1
    bpk[C + 1, _b_off("f1")] = 1.0

    emb = np.asarray(embedding, f)[0]            # (512, 80)
    e2 = (emb.astype(np.float64) ** 2).sum(1)
    ew = (emb.astype(np.float64) @ np.asarray(w_lin, f).T.astype(np.float64))
    epk3 = np.zeros((128, 3 * NK), f)
    for ti in range(3):
        epk3[:C, NK * ti:NK * (ti + 1)] = emb.T
        epk3[C, NK * ti:NK * (ti + 1)] = -0.5 * e2
    epk3[C + 1, NK:2 * NK] = ew[:, 0] / BIG
    epk3[C + 1, 2 * NK:3 * NK] = ew[:, 1] / BIG
    epk3 = epk3.astype(bf)
    z48 = np.zeros((48, 4104), dtype=bf)
    ztail = np.zeros((2, 32), f)
    ztail[:, 24:] = NEG
    wlt = np.zeros((128, 2), dtype=bf)
    wlt[:C] = np.ascontiguousarray(np.asarray(w_lin, f).T).astype(bf)
    b_lin = np.asarray(b_lin, f)
    blt = np.array([[b_lin[0], b_lin[1], b_lin[0], b_lin[1]]], f)

    maps = []
    for c in range(NCORES):
        sl = slice(SPC * c, SPC * (c + 1))
        maps.append({
            "se": se[sl], "so": so[sl], "qe": qe[sl], "qo": qo[sl],
            "wpk": wpk, "bpk": bpk, "epk3": epk3, "wlt": wlt, "blt": blt,
            "z48": z48, "ztail": ztail,
        })
    return maps


def kernel(**inputs):
    nc = _get_nc()
    maps = prep_inputs(**inputs)
    res = run_bass_kernel_spmd(nc, maps, core_ids=list(range(NCORES)))
    out = np.concatenate([r["out"].reshape(SPC, 2) for r in res.results],
                         axis=0)
    return out.astype(np.float32)


if __name__ == "__main__":
    import reference
    inputs = {k: np.asarray(v) for k, v in reference.setup_inputs().items()}
    got = kernel(**inputs)
    print(got)


# revision 24
# speedup vs baseline: 1.1763x; 1.0458x over previous
"""AudioFinder Trainium2 kernel.

Data parallel over batch: 16 samples -> 8 cores x 2 samples.

Per-core pipeline (bf16 matmuls / f32 psum, both samples interleaved
layer-by-layer so one sample's matmuls fill the other's pipeline-latency
bubbles on the in-order engine queues):
  1. Both query encoders (T=2048 -> 504), layers interleaved; v =
     w_lin @ enc_q; the tiled-x4 + pad row [1,2048] is bounced through
     DRAM into [128,16] (t = p + 128*b) while the search encoders run.
  2. Both search encoders (T=8192 -> 2040), layers interleaved, with
     the f0/f1 head fused into layer 3 and VQ 128-t blocks emitted as
     separate scheduler units paced between conv chunks so the DVE
     max-reduces never back up the PE's PSUM banks.
  3. VQ per 128-t block: three bf16 matmuls (enc block stationary)
     against epk3's three NK-column blocks, DVE max-reduce each:
     scores s[t,k] = enc_s[t]@emb[k] - |emb[k]|^2/2 (enc rows 80/81
     const 1.0; epk3 row 80 carries -|e|^2/2 shared, row 81 carries
     0 / ew0/BIG / ew1/BIG with ew = emb @ w_lin.T):
       u_j[t] = max_k (s[t,k] + ew[k,j]/BIG),  m[t] = max_k s[t,k]
       => (u_j - m)*BIG = ew[argmax_k s, j]   (fp32-psum exact)
  4. z = (u-m)*BIG + vt in [128,16]; max over free dim on DVE, across
     partitions on Pool; out = tanh(max z + b_lin).

All 1x1-conv biases are folded (host-side) into the next layer's tap
biases / f0's bias, so the residual writers are plain psum+residual
adds on DVE and layer-0's writer is a pair of ACT copies.  Every
matmul keeps a 128-partition stationary operand (zero rows beyond 80)
so FWL stays enabled; x2/xf ring rows 80-127 are zeroed once at start.
Gated tanh*sigmoid on ACT, gate product on Pool.
"""

import numpy as np
import ml_dtypes

import concourse.bacc as bacc
import concourse.mybir as mybir
import concourse.tile as tile
from concourse.bass_utils import run_bass_kernel_spmd

F32 = mybir.dt.float32
BF16 = mybir.dt.bfloat16
AF = mybir.ActivationFunctionType
OP = mybir.AluOpType
AX = mybir.AxisListType

NCORES = 8
SPC = 2          # samples per core
C = 80
NK = 512         # codebook size
BIG = 1024.0
NEG = -1e30
CH = 512         # chunk (free-dim) size

# layer geometry
GEO_SEARCH = dict(T0h=4096, T1=4095, E1=2048, O1=2047, T2=2046, T3=2043, T4=2040)
GEO_QUERY = dict(T0h=1024, T1=1023, E1=512, O1=511, T2=510, T3=507, T4=504)

# wpack layout: layer-major [a_i(4 taps), g_i(4 taps), w1x1_i] x 4, f0, f1
LBLK = 720  # 4*80 + 4*80 + 80

def _w_off(kind, i, j=0):
    if kind == "a":
        return LBLK * i + C * j
    if kind == "g":
        return LBLK * i + 320 + C * j
    if kind == "1":
        return LBLK * i + 640
    if kind == "f0":
        return 2880
    if kind == "f1":
        return 2960
    raise KeyError(kind)


M_F1 = 82  # f1 conv emits 80 real channels + two const-1 channels


WPACK_COLS = 3042
# bias pack columns: ba0..3, bg0..3, b10..3, bf0, bf1
def _b_off(kind, i=0):
    return {"a": i, "g": 4 + i, "1": 8 + i, "f0": 12, "f1": 13}[kind]


def _build():
    nc = bacc.Bacc("TRN2", target_bir_lowering=False, debug=False,
                   num_devices=NCORES)
    d_se = nc.dram_tensor("se", [SPC, 128, 4096], BF16, kind="ExternalInput")
    d_so = nc.dram_tensor("so", [SPC, 128, 4096], BF16, kind="ExternalInput")
    d_qe = nc.dram_tensor("qe", [SPC, 128, 1024], BF16, kind="ExternalInput")
    d_qo = nc.dram_tensor("qo", [SPC, 128, 1024], BF16, kind="ExternalInput")
    d_wpk = nc.dram_tensor("wpk", [128, WPACK_COLS], BF16, kind="ExternalInput")
    d_bpk = nc.dram_tensor("bpk", [M_F1, 14], F32, kind="ExternalInput")
    d_epk3 = nc.dram_tensor("epk3", [128, 3 * NK], BF16, kind="ExternalInput")
    d_z = nc.dram_tensor("z48", [48, 4104], BF16, kind="ExternalInput")
    d_ztail = nc.dram_tensor("ztail", [2, 32], F32, kind="ExternalInput")
    d_wlt = nc.dram_tensor("wlt", [128, 2], BF16, kind="ExternalInput")
    d_blt = nc.dram_tensor("blt", [1, 4], F32, kind="ExternalInput")
    d_out = nc.dram_tensor("out", [1, 4], F32, kind="ExternalOutput")
    d_zbuf = nc.dram_tensor("zbuf", [2 * SPC, 2048], F32)

    with tile.TileContext(nc) as tc:
        with (
            tc.tile_pool(name="sb", bufs=1) as sb,
            tc.tile_pool(name="ps", bufs=2, space="PSUM") as ps,
        ):
            # ---- startup DMAs, ordered so layer-0 work can start early ----
            wpk = sb.tile([128, WPACK_COLS], BF16, tag="wpk")
            nc.sync.dma_start(wpk[:, :LBLK], d_wpk[:, :LBLK])
            bpk = sb.tile([M_F1, 14], F32, tag="bpk")
            nc.sync.dma_start(bpk[:], d_bpk[:])
            # zero rows 80-127 of the x2/xf ring buffers once, so the
            # 1x1/f1 matmuls can use 128-partition operands (FWL on)
            for _ in range(4):
                x2t = sb.tile([128, CH], BF16, tag="x2", bufs=4)
                nc.vector.memset(x2t[64:, :], 0.0)
                xft = sb.tile([128, CH], BF16, tag="xf", bufs=4)
                nc.vector.memset(xft[64:, :], 0.0)

            def wsl(kind, i, j=0, rows=128):
                off = _w_off(kind, i, j)
                return wpk[:rows, off:off + C]

            def bap(kind, i=0):
                o = _b_off(kind, i)
                n = M_F1 if kind == "f1" else C
                return bpk[:n, o:o + 1]

            def wide_chunk_segs(i, taps, T_out, write_out, c0):
                """One CH-column chunk of wide conv layer i, split into
                PE segments so the scheduler can software-pipeline the
                in-order PE queue (the 1x1 waits ~2us for tanh/sig/mul;
                another unit's taps fill that hole)."""
                N = min(CH, T_out - c0)
                Nmm = N + (N & 1)
                cell = {}

                def seg0():
                    aps = ps.tile([C, Nmm], F32, tag="aps")
                    gps = ps.tile([C, Nmm], F32, tag="gps")
                    for half, pt in (("a", aps), ("g", gps)):
                        for j, (src, off) in enumerate(taps):
                            nc.tensor.matmul(
                                pt[:], wsl(half, i, j),
                                src[:, off + c0: off + c0 + Nmm],
                                start=(j == 0), stop=(j == 3))
                    ta = sb.tile([C, Nmm], BF16, tag="ta", bufs=4)
                    sg = sb.tile([C, Nmm], BF16, tag="sg", bufs=4)
                    nc.scalar.activation(ta[:], aps[:], AF.Tanh,
                                         bias=bap("a", i))
                    nc.scalar.activation(sg[:], gps[:], AF.Sigmoid,
                                         bias=bap("g", i))
                    x2 = sb.tile([128, Nmm], BF16, tag="x2", bufs=4)
                    # L0/L1 run in the VQ-free window where DVE is idle
                    # (and 3x faster than Pool); L2/L3 stay on Pool to
                    # keep DVE clear for the VQ max-reduces
                    if i <= 1:
                        nc.vector.tensor_mul(x2[:C], ta[:], sg[:])
                    else:
                        nc.gpsimd.tensor_mul(x2[:C], ta[:], sg[:])
                    cell["x2"] = x2

                def seg1():
                    xps = ps.tile([C, Nmm], F32, tag="xps")
                    nc.tensor.matmul(xps[:], wsl("1", i), cell["x2"][:],
                                     start=True, stop=True)
                    return write_out(c0, N, xps[:, :N])

                return [seg0, seg1]

            def encoder_units(s, g, is_query):
                """Chunk-level emitters for the scheduler.

                Layer 3 fuses the f0/f1 head; VQ blocks (search) are
                separate units paced by the scheduler."""
                T0h, T1 = g["T0h"], g["T1"]
                E1, O1 = g["E1"], g["O1"]
                T2, T3, T4 = g["T2"], g["T3"], g["T4"]
                d_e, d_o = (d_qe, d_qo) if is_query else (d_se, d_so)
                sfx = f"{'q' if is_query else 's'}{s}"
                st = {}

                def alloc0():
                    x0e = sb.tile([128, T0h + 8], BF16, tag=f"x0e{sfx}")
                    x0o = sb.tile([128, T0h + 8], BF16, tag=f"x0o{sfx}")
                    nc.vector.memset(x0e[:, T0h:], 0.0)
                    nc.vector.memset(x0o[:, T0h:], 0.0)
                    st.update(x0e=x0e, x0o=x0o)

                def load_dma(c0, n):
                    n = min(n, T0h - c0)
                    nc.sync.dma_start(st["x0e"][:, c0:c0 + n],
                                      d_e[s, :, c0:c0 + n])
                    nc.sync.dma_start(st["x0o"][:, c0:c0 + n],
                                      d_o[s, :, c0:c0 + n])

                def alloc_a():
                    x1e = sb.tile([128, E1 + 8], BF16, tag=f"x1e{sfx}")
                    x1o = sb.tile([128, O1 + 8], BF16, tag=f"x1o{sfx}")
                    nc.vector.memset(x1e[:, E1:], 0.0)
                    nc.vector.memset(x1o[:, O1:], 0.0)
                    nc.sync.dma_start(x1e[C:, :E1], d_z[:, :E1])
                    nc.sync.dma_start(x1o[C:, :O1], d_z[:, :O1])
                    st.update(x1e=x1e, x1o=x1o)

                def alloc_b():
                    x2f = sb.tile([128, T2 + 8], BF16, tag=f"x2f{sfx}")
                    nc.vector.memset(x2f[:, T2:], 0.0)
                    nc.sync.dma_start(x2f[C:, :T2], d_z[:, :T2])
                    x3f = sb.tile([128, T3 + 8], BF16, tag=f"x3f{sfx}")
                    nc.vector.memset(x3f[:, T3:], 0.0)
                    nc.sync.dma_start(x3f[C:, :T3], d_z[:, :T3])
                    st.update(x2f=x2f, x3f=x3f)

                def alloc_c():
                    x4f = sb.tile([128, T4 + 8], BF16, tag=f"x4f{sfx}")
                    nc.vector.memset(x4f[:, T4:], 0.0)
                    nc.sync.dma_start(x4f[C:, :T4], d_z[:, :T4])
                    # f1(relu(f0(x))); rows 80/81 of enc are const 1.0
                    # (zero weights, bias 1) for the VQ score offsets.
                    enc = sb.tile([128, T4], BF16, tag=f"enc{sfx}")
                    nc.sync.dma_start(enc[M_F1:, :], d_z[:128 - M_F1, :T4])
                    st.update(x4f=x4f, enc=enc)

                def w0(c0, N, xps):
                    ne, no = (N + 1) // 2, N // 2
                    h = c0 // 2
                    nc.scalar.activation(
                        st["x1e"][:C, h:h + ne], xps[:, 0:N:2], AF.Copy)
                    nc.vector.tensor_copy(
                        st["x1o"][:C, h:h + no], xps[:, 1:N:2])

                def w1(c0, N, xps):
                    nc.vector.tensor_add(
                        st["x2f"][:C, c0:c0 + N], xps,
                        st["x1o"][:C, c0 + 1:c0 + 1 + N])

                def w2(c0, N, xps):
                    nc.vector.tensor_add(
                        st["x3f"][:C, c0:c0 + N], xps,
                        st["x2f"][:C, c0 + 3:c0 + 3 + N])

                def w3(c0, N, xps):
                    x4f, x3f, enc = st["x4f"], st["x3f"], st["enc"]
                    nc.vector.tensor_add(
                        x4f[:C, c0:c0 + N], xps,
                        x3f[:C, c0 + 3:c0 + 3 + N])
                    cf = {}

                    def seg_f0():
                        p0 = ps.tile([C, N], F32, tag="xps")
                        nc.tensor.matmul(p0[:], wsl("f0", 0),
                                         x4f[:, c0:c0 + N],
                                         start=True, stop=True)
                        xf = sb.tile([128, N], BF16, tag="xf", bufs=4)
                        nc.scalar.activation(xf[:C], p0[:], AF.Relu,
                                             bias=bap("f0"))
                        cf["xf"] = xf

                    def seg_f1():
                        p1 = ps.tile([M_F1, N], F32, tag="xps")
                        nc.tensor.matmul(p1[:], wpk[:, 2960:2960 + M_F1],
                                         cf["xf"][:], start=True, stop=True)
                        nc.scalar.activation(enc[:M_F1, c0:c0 + N], p1[:],
                                             AF.Identity, bias=bap("f1"))

                    return [seg_f0, seg_f1]

                Ts = [T1, T2, T3, T4]
                writers = [w0, w1, w2, w3]

                def taps(i):
                    if i == 0:
                        return [(st["x0e"], 0), (st["x0o"], 0),
                                (st["x0e"], 1), (st["x0o"], 1)]
                    if i == 1:
                        return [(st["x1e"], 0), (st["x1o"], 0),
                                (st["x1e"], 1), (st["x1o"], 1)]
                    x = st["x2f"] if i == 2 else st["x3f"]
                    return [(x, 0), (x, 1), (x, 2), (x, 3)]

                def emit_segs(i, c):
                    return wide_chunk_segs(i, taps(i), Ts[i], writers[i],
                                           c * CH)

                nch = [-(-t // CH) for t in Ts]
                ins = [E1, T2, T3]

                def req(i, c):
                    """Chunks of layer i-1 needed before chunk c of layer i."""
                    per = CH // 2 if i == 1 else CH
                    need = min(CH * c + 515, ins[i - 1])
                    return min(nch[i - 1], -(-need // per))

                return dict(alloc0=alloc0, load_dma=load_dma, alloc_a=alloc_a,
                            alloc_b=alloc_b, alloc_c=alloc_c,
                            emit_segs=emit_segs, nch=nch, req=req, st=st)

            def run_sched(encs, prios, vqs=None, post_l3=None):
                """Software-pipelined emission: each conv unit is a list
                of PE segments (taps / 1x1 / f0 / f1); a segment is only
                emitted one turn after its predecessor so another unit's
                taps fill the PE queue between dependent matmuls.  VQ
                (matmul, max-reduce) singles are paced in between."""
                sprog = [[0] * 4 for _ in encs]   # units started
                wprog = [[0] * 4 for _ in encs]   # units fully emitted
                conv_rem = sum(sum(e["nch"]) for e in encs)
                segq = []                          # [closures, sidx, i, turn]
                vprog = {sidx: 0 for sidx in (vqs or {})}
                vtot = {sidx: 3 * v["nblk"] for sidx, v in (vqs or {}).items()}
                turn = 0

                def unit_done(sidx, i):
                    wprog[sidx][i] += 1
                    if (i == 3 and wprog[sidx][3] == encs[sidx]["nch"][3]
                            and post_l3):
                        post_l3(sidx)

                def run_seg(entry):
                    closures, sidx, i, _ = entry
                    more = closures.pop(0)()
                    if more:
                        closures.extend(more)
                    if closures:
                        segq.append([closures, sidx, i, turn])
                    else:
                        unit_done(sidx, i)

                def try_new(group):
                    nonlocal conv_rem
                    for k in range(len(group)):
                        sidx = group[(turn + k) % len(group)]
                        e, p = encs[sidx], sprog[sidx]
                        for i in (3, 2, 1, 0):
                            if p[i] >= e["nch"][i]:
                                continue
                            if i > 0 and wprog[sidx][i - 1] < e["req"](i, p[i]):
                                continue
                            segs = e["emit_segs"](i, p[i])
                            p[i] += 1
                            conv_rem -= 1
                            run_seg([segs, sidx, i, turn])
                            return True
                    return False

                def emit_vq(budget):
                    done = 0
                    keys = sorted(vqs)
                    for k in range(len(keys)):
                        if done >= budget:
                            break
                        sidx = keys[(turn + k) % len(keys)]
                        v = vqs[sidx]
                        while vprog[sidx] < vtot[sidx] and done < budget:
                            b, ti = divmod(vprog[sidx], 3)
                            if wprog[sidx][3] < v["req"](b):
                                break
                            v["emit_one"](b, ti)
                            vprog[sidx] += 1
                            done += 1
                    return done

                while (conv_rem or segq or
                       any(vprog[s] < vtot[s] for s in vprog)):
                    emitted = 0
                    # cap in-flight multi-segment units so the ring
                    # buffers (depth 4) can never cycle-deadlock
                    if len(segq) < 3:
                        for group in prios:
                            if try_new(group):
                                emitted += 1
                                break
                    if segq and segq[0][3] < turn:
                        run_seg(segq.pop(0))
                        emitted += 1
                        # drain backlogged segments faster at the conv tail
                        if len(segq) >= 2 and segq[0][3] < turn:
                            run_seg(segq.pop(0))
                            emitted += 1
                    if vqs:
                        emitted += emit_vq(3 if emitted else 8)
                    if not emitted and segq:
                        run_seg(segq.pop(0))
                        emitted = 1
                    turn += 1
                    assert emitted, "scheduler deadlock"

            # ---- encoders: query inputs first; search streams behind and
            # runs in the middle; query sample 1 is held back to overlap
            # the VQ drain at the end ----
            qencs = [encoder_units(s, GEO_QUERY, True) for s in range(SPC)]
            sencs = [encoder_units(s, GEO_SEARCH, False) for s in range(SPC)]
            for e in qencs + sencs:
                e["alloc0"]()
            for c0 in (0, 512):
                for e in qencs:
                    e["load_dma"](c0, 512)
            for e in qencs:
                e["alloc_a"]()
                e["alloc_b"]()
                e["alloc_c"]()
            # remaining static tables arrive behind the query inputs
            nc.sync.dma_start(wpk[:, LBLK:], d_wpk[:, LBLK:])
            epk3 = sb.tile([128, 3 * NK], BF16, tag="epk3")
            nc.sync.dma_start(epk3[:], d_epk3[:])
            wlt = sb.tile([128, 2], BF16, tag="wlt")
            nc.sync.dma_start(wlt[:], d_wlt[:])
            brow = sb.tile([1, 4], F32, tag="brow")
            nc.sync.dma_start(brow[:], d_blt[:])
            ztail = sb.tile([2, 32], F32, tag="ztail")
            nc.sync.dma_start(ztail[:], d_ztail[:])
            # search inputs interleaved with their zero-fills, ordered so
            # each layer's zeros land before that layer can be scheduled
            for e in sencs:
                e["load_dma"](0, 1024)
            for e in sencs:
                e["alloc_a"]()
            for e in sencs:
                e["load_dma"](1024, 1024)
            for e in sencs:
                e["alloc_b"]()
            for e in sencs:
                e["load_dma"](2048, 1024)
            for e in sencs:
                e["alloc_c"]()
            for e in sencs:
                e["load_dma"](3072, 1024)

            T4s = GEO_SEARCH["T4"]
            accs = {}
            for s in range(SPC):
                mt = sb.tile([128, 16], F32, tag=f"mt{s}")
                u0t = sb.tile([128, 16], F32, tag=f"u0t{s}")
                u1t = sb.tile([128, 16], F32, tag=f"u1t{s}")
                nc.vector.memset(mt[:], NEG)
                nc.vector.memset(u0t[:], NEG)
                nc.vector.memset(u1t[:], NEG)
                accs[s] = (mt, u0t, u1t)

            import concourse.bass_isa as bass_isa
            # warm the Q7 IRAM for partition_all_reduce early so the
            # final reduce doesn't pay the ~6us ext-isa load at the tail
            zwarm = sb.tile([128, 16], F32, tag="zwarm")
            nc.gpsimd.partition_all_reduce(zwarm[:], accs[0][0][:],
                                           channels=128,
                                           reduce_op=bass_isa.ReduceOp.max)

            zred = sb.tile([128, 4], F32, tag="zred")
            vts = {}

            def emit_vpath(s):
                """v = w_lin @ enc_q, tiled x4 through DRAM into [128,16]."""
                enc_q = qencs[s]["st"]["enc"]
                vps = ps.tile([128, NK], F32, tag="vq")
                nc.tensor.matmul(vps[:2, :504], wlt[:], enc_q[:, :504],
                                 start=True, stop=True)
                vrow = sb.tile([2, 504], F32, tag="vrow", bufs=2)
                nc.scalar.activation(vrow[:], vps[:2, :504], AF.Copy)
                for k in range(4):
                    nc.sync.dma_start(
                        d_zbuf[2 * s:2 * s + 2, 504 * k:504 * (k + 1)],
                        vrow[:])
                nc.sync.dma_start(d_zbuf[2 * s:2 * s + 2, 2016:], ztail[:])
                vt0 = sb.tile([128, 16], F32, tag=f"vt{2 * s}")
                vt1 = sb.tile([128, 16], F32, tag=f"vt{2 * s + 1}")
                nc.sync.dma_start(
                    vt0[:], d_zbuf[2 * s].rearrange("(b p) -> p b", p=128))
                nc.sync.dma_start(
                    vt1[:], d_zbuf[2 * s + 1].rearrange("(b p) -> p b", p=128))
                vts[s] = (vt0, vt1)

            def make_vq(s):
                tgts = accs[s]
                st = sencs[s]["st"]

                def emit_one(b, ti):
                    t0 = 128 * b
                    P = min(128, T4s - t0)
                    sps = ps.tile([128, NK], F32, tag="vq")
                    nc.tensor.matmul(
                        sps[:P, :], st["enc"][:, t0:t0 + P],
                        epk3[:, NK * ti:NK * (ti + 1)],
                        start=True, stop=True)
                    nc.vector.tensor_reduce(
                        tgts[ti][:P, b:b + 1], sps[:P, :], axis=AX.X,
                        op=OP.max)

                def req(b):
                    return min(sencs[s]["nch"][3], -(-(128 * (b + 1)) // CH))

                return dict(emit_one=emit_one, req=req, nblk=16)

            def post_l3(sidx):
                if sidx < SPC:          # query sample finished its encoder
                    emit_vpath(sidx)

            vqs = {SPC + s: make_vq(s) for s in range(SPC)}
            run_sched(qencs + sencs,
                      prios=[[0], [2, 3], [1]], vqs=vqs, post_l3=post_l3)

            # ---- z = (u - m)*BIG + vt, reduce ----
            for s in range(SPC):
                mt, u0t, u1t = accs[s]
                for j, ut in ((0, u0t), (1, u1t)):
                    zt = sb.tile([128, 16], F32, tag="zt", bufs=2)
                    nc.vector.tensor_sub(zt[:], ut[:], mt[:])
                    nc.vector.scalar_tensor_tensor(
                        out=zt[:], in0=zt[:], scalar=BIG, in1=vts[s][j][:],
                        op0=OP.mult, op1=OP.add)
                    nc.vector.tensor_reduce(
                        zred[:, 2 * s + j:2 * s + j + 1], zt[:], axis=AX.X,
                        op=OP.max)

            zar = sb.tile([128, 4], F32, tag="zar")
            nc.gpsimd.partition_all_reduce(zar[:], zred[:], channels=128,
                                           reduce_op=bass_isa.ReduceOp.max)
            zrow = sb.tile([1, 4], F32, tag="zrow")
            nc.vector.tensor_add(zrow[:], zar[0:1, :], brow[:])
            outv = sb.tile([1, 4], F32, tag="outv")
            nc.scalar.activation(outv[:], zrow[:], AF.Tanh)
            nc.sync.dma_start(d_out[:], outv[:])

    nc.finalize()
    return nc


_NC_CACHE = None


def _get_nc():
    global _NC_CACHE
    if _NC_CACHE is None:
        _NC_CACHE = _build()
    return _NC_CACHE


def prep_inputs(search, query, w_wide, b_wide, w_1x1, b_1x1, w_f0, b_f0,
                w_f1, b_f1, embedding, w_lin, b_lin):
    """Host-side packing -> list of per-core input maps (bf16 operands)."""
    f = np.float32
    bf = ml_dtypes.bfloat16
    search = np.asarray(search, f)
    query = np.asarray(query, f)

    def pad128(a):
        # (N, T, C) -> (N, 128, T) channel-major, rows C..127 zero
        n, t, _ = a.shape
        out = np.zeros((n, 128, t), dtype=bf)
        out[:, :C, :] = a.transpose(0, 2, 1).astype(bf)
        return out

    se = pad128(search[:, 0::2, :])
    so = pad128(search[:, 1::2, :])
    qe = pad128(query[:, 0::2, :])
    qo = pad128(query[:, 1::2, :])

    w_wide = np.asarray(w_wide, f)
    w_1x1 = np.asarray(w_1x1, f)
    b_wide = np.asarray(b_wide, f)
    b_1x1 = np.asarray(b_1x1, f)
    cols = []
    for i in range(4):
        for j in range(4):
            cols.append(w_wide[i, :C, :, j].T)     # a taps
        for j in range(4):
            cols.append(w_wide[i, C:, :, j].T)     # g taps
        cols.append(w_1x1[i, :, :, 0].T)
    cols.append(np.asarray(w_f0, f)[:, :, 0].T)
    wf1 = np.zeros((C, M_F1), f)
    wf1[:, :C] = np.asarray(w_f1, f)[:, :, 0].T   # cols 80/81 stay zero
    cols.append(wf1)
    wpk80 = np.ascontiguousarray(np.concatenate(cols, axis=1))
    assert wpk80.shape == (C, WPACK_COLS)
    wpk = np.zeros((128, WPACK_COLS), dtype=bf)
    wpk[:C] = wpk80.astype(bf)

    # cumulative 1x1 biases folded into next layer's tap biases / f0 bias
    cum = np.cumsum(b_1x1, axis=0)                 # cum[i] = b0+..+bi
    bcols = []
    for i in range(4):                             # a-tap biases
        ba = b_wide[i, :C].copy()
        if i > 0:
            ba += w_wide[i, :C].sum(axis=2) @ cum[i - 1]
        bcols.append(ba)
    for i in range(4):                             # g-tap biases
        bg = b_wide[i, C:].copy()
        if i > 0:
            bg += w_wide[i, C:].sum(axis=2) @ cum[i - 1]
        bcols.append(bg)
    bcols += [np.zeros(C, f)] * 4                  # (1x1 biases folded away)
    bf0 = np.asarray(b_f0, f) + np.asarray(w_f0, f)[:, :, 0] @ cum[3]
    bcols += [bf0, np.asarray(b_f1, f)]
    bpk = np.zeros((M_F1, 14), f)
    bpk[:C] = np.stack(bcols, axis=1)
    bpk[C, _b_off("f1")] = 1.0     # f1 rows 80/81 = 0*x + 1.0 -> const-1
    bpk[C + 1, _b_off("f1")] = 1.0

    emb = np.asarray(embedding, f)[0]            # (512, 80)
    e2 = (emb.astype(np.float64) ** 2).sum(1)
    ew = (emb.astype(np.float64) @ np.asarray(w_lin, f).T.astype(np.float64))
    epk3 = np.zeros((128, 3 * NK), f)
    for ti in range(3):
        epk3[:C, NK * ti:NK * (ti + 1)] = emb.T
        epk3[C, NK * ti:NK * (ti + 1)] = -0.5 * e2
    epk3[C + 1, NK:2 * NK] = ew[:, 0] / BIG
    epk3[C + 1, 2 * NK:3 * NK] = ew[:, 1] / BIG
    epk3 = epk3.astype(bf)
    z48 = np.zeros((48, 4104), dtype=bf)
    ztail = np.zeros((2, 32), f)
    ztail[:, 24:] = NEG
    wlt = np.zeros((128, 2), dtype=bf)
    wlt[:C] = np.ascontiguousarray(np.asarray(w_lin, f).T).astype(bf)
    b_lin = np.asarray(b_lin, f)
    blt = np.array([[b_lin[0], b_lin[1], b_lin[0], b_lin[1]]], f)

    maps = []
    for c in range(NCORES):
        sl = slice(SPC * c, SPC * (c + 1))
        maps.append({
            "se": se[sl], "so": so[sl], "qe": qe[sl], "qo": qo[sl],
            "wpk": wpk, "bpk": bpk, "epk3": epk3, "wlt": wlt, "blt": blt,
            "z48": z48, "ztail": ztail,
        })
    return maps


def kernel(**inputs):
    nc = _get_nc()
    maps = prep_inputs(**inputs)
    res = run_bass_kernel_spmd(nc, maps, core_ids=list(range(NCORES)))
    out = np.concatenate([r["out"].reshape(SPC, 2) for r in res.results],
                         axis=0)
    return out.astype(np.float32)


if __name__ == "__main__":
    import reference
    inputs = {k: np.asarray(v) for k, v in reference.setup_inputs().items()}
    got = kernel(**inputs)
    print(got)


# revision 25
# speedup vs baseline: 1.1804x; 1.0035x over previous
"""AudioFinder Trainium2 kernel.

Data parallel over batch: 16 samples -> 8 cores x 2 samples.

Per-core pipeline (bf16 matmuls / f32 psum, both samples interleaved
layer-by-layer so one sample's matmuls fill the other's pipeline-latency
bubbles on the in-order engine queues):
  1. Both query encoders (T=2048 -> 504), layers interleaved; v =
     w_lin @ enc_q; the tiled-x4 + pad row [1,2048] is bounced through
     DRAM into [128,16] (t = p + 128*b) while the search encoders run.
  2. Both search encoders (T=8192 -> 2040), layers interleaved, with
     the f0/f1 head fused into layer 3 and VQ 128-t blocks emitted as
     separate scheduler units paced between conv chunks so the DVE
     max-reduces never back up the PE's PSUM banks.
  3. VQ per 128-t block: three bf16 matmuls (enc block stationary)
     against epk3's three NK-column blocks, DVE max-reduce each:
     scores s[t,k] = enc_s[t]@emb[k] - |emb[k]|^2/2 (enc rows 80/81
     const 1.0; epk3 row 80 carries -|e|^2/2 shared, row 81 carries
     0 / ew0/BIG / ew1/BIG with ew = emb @ w_lin.T):
       u_j[t] = max_k (s[t,k] + ew[k,j]/BIG),  m[t] = max_k s[t,k]
       => (u_j - m)*BIG = ew[argmax_k s, j]   (fp32-psum exact)
  4. z = (u-m)*BIG + vt in [128,16]; max over free dim on DVE, across
     partitions on Pool; out = tanh(max z + b_lin).

All 1x1-conv biases are folded (host-side) into the next layer's tap
biases / f0's bias, so the residual writers are plain psum+residual
adds on DVE and layer-0's writer is a pair of ACT copies.  Every
matmul keeps a 128-partition stationary operand (zero rows beyond 80)
so FWL stays enabled; x2/xf ring rows 80-127 are zeroed once at start.
Gated tanh*sigmoid on ACT, gate product on Pool.
"""

import numpy as np
import ml_dtypes

import concourse.bacc as bacc
import concourse.mybir as mybir
import concourse.tile as tile
from concourse.bass_utils import run_bass_kernel_spmd

F32 = mybir.dt.float32
BF16 = mybir.dt.bfloat16
AF = mybir.ActivationFunctionType
OP = mybir.AluOpType
AX = mybir.AxisListType

NCORES = 8
SPC = 2          # samples per core
C = 80
NK = 512         # codebook size
BIG = 1024.0
NEG = -1e30
CH = 512         # chunk (free-dim) size

# layer geometry
GEO_SEARCH = dict(T0h=4096, T1=4095, E1=2048, O1=2047, T2=2046, T3=2043, T4=2040)
GEO_QUERY = dict(T0h=1024, T1=1023, E1=512, O1=511, T2=510, T3=507, T4=504)

# wpack layout: layer-major [a_i(4 taps), g_i(4 taps), w1x1_i] x 4, f0, f1
LBLK = 720  # 4*80 + 4*80 + 80

def _w_off(kind, i, j=0):
    if kind == "a":
        return LBLK * i + C * j
    if kind == "g":
        return LBLK * i + 320 + C * j
    if kind == "1":
        return LBLK * i + 640
    if kind == "f0":
        return 2880
    if kind == "f1":
        return 2960
    raise KeyError(kind)


M_F1 = 82  # f1 conv emits 80 real channels + two const-1 channels


WPACK_COLS = 3042
# bias pack columns: ba0..3, bg0..3, b10..3, bf0, bf1
def _b_off(kind, i=0):
    return {"a": i, "g": 4 + i, "1": 8 + i, "f0": 12, "f1": 13}[kind]


def _build():
    nc = bacc.Bacc("TRN2", target_bir_lowering=False, debug=False,
                   num_devices=NCORES)
    d_se = nc.dram_tensor("se", [SPC, 128, 4096], BF16, kind="ExternalInput")
    d_so = nc.dram_tensor("so", [SPC, 128, 4096], BF16, kind="ExternalInput")
    d_qe = nc.dram_tensor("qe", [SPC, 128, 1024], BF16, kind="ExternalInput")
    d_qo = nc.dram_tensor("qo", [SPC, 128, 1024], BF16, kind="ExternalInput")
    d_wpk = nc.dram_tensor("wpk", [128, WPACK_COLS], BF16, kind="ExternalInput")
    d_bpk = nc.dram_tensor("bpk", [M_F1, 14], F32, kind="ExternalInput")
    d_epk3 = nc.dram_tensor("epk3", [128, 3 * NK], BF16, kind="ExternalInput")
    d_z = nc.dram_tensor("z48", [48, 4104], BF16, kind="ExternalInput")
    d_ztail = nc.dram_tensor("ztail", [2, 32], F32, kind="ExternalInput")
    d_wlt = nc.dram_tensor("wlt", [128, 2], BF16, kind="ExternalInput")
    d_blt = nc.dram_tensor("blt", [1, 4], F32, kind="ExternalInput")
    d_out = nc.dram_tensor("out", [1, 4], F32, kind="ExternalOutput")
    d_zbuf = nc.dram_tensor("zbuf", [2 * SPC, 2048], F32)

    with tile.TileContext(nc) as tc:
        with (
            tc.tile_pool(name="sb", bufs=1) as sb,
            tc.tile_pool(name="ps", bufs=2, space="PSUM") as ps,
        ):
            # ---- startup DMAs, ordered so layer-0 work can start early ----
            wpk = sb.tile([128, WPACK_COLS], BF16, tag="wpk")
            nc.sync.dma_start(wpk[:, :LBLK], d_wpk[:, :LBLK])
            bpk = sb.tile([M_F1, 14], F32, tag="bpk")
            nc.sync.dma_start(bpk[:], d_bpk[:])
            # zero rows 80-127 of the x2/xf ring buffers once, so the
            # 1x1/f1 matmuls can use 128-partition operands (FWL on)
            for _ in range(4):
                x2t = sb.tile([128, CH], BF16, tag="x2", bufs=4)
                nc.vector.memset(x2t[64:, :], 0.0)
                xft = sb.tile([128, CH], BF16, tag="xf", bufs=4)
                nc.vector.memset(xft[64:, :], 0.0)

            def wsl(kind, i, j=0, rows=128):
                off = _w_off(kind, i, j)
                return wpk[:rows, off:off + C]

            def bap(kind, i=0):
                o = _b_off(kind, i)
                n = M_F1 if kind == "f1" else C
                return bpk[:n, o:o + 1]

            def wide_chunk_segs(i, taps, T_out, write_out, c0):
                """One CH-column chunk of wide conv layer i, split into
                PE segments so the scheduler can software-pipeline the
                in-order PE queue (the 1x1 waits ~2us for tanh/sig/mul;
                another unit's taps fill that hole)."""
                N = min(CH, T_out - c0)
                Nmm = N + (N & 1)
                cell = {}

                def seg0():
                    aps = ps.tile([C, Nmm], F32, tag="aps")
                    gps = ps.tile([C, Nmm], F32, tag="gps")
                    for half, pt in (("a", aps), ("g", gps)):
                        for j, (src, off) in enumerate(taps):
                            nc.tensor.matmul(
                                pt[:], wsl(half, i, j),
                                src[:, off + c0: off + c0 + Nmm],
                                start=(j == 0), stop=(j == 3))
                    ta = sb.tile([C, Nmm], BF16, tag="ta", bufs=4)
                    sg = sb.tile([C, Nmm], BF16, tag="sg", bufs=4)
                    nc.scalar.activation(ta[:], aps[:], AF.Tanh,
                                         bias=bap("a", i))
                    nc.scalar.activation(sg[:], gps[:], AF.Sigmoid,
                                         bias=bap("g", i))
                    x2 = sb.tile([128, Nmm], BF16, tag="x2", bufs=4)
                    # L0/L1 run in the VQ-free window where DVE is idle
                    # (and 3x faster than Pool); L2/L3 stay on Pool to
                    # keep DVE clear for the VQ max-reduces
                    if i <= 1:
                        nc.vector.tensor_mul(x2[:C], ta[:], sg[:])
                    else:
                        nc.gpsimd.tensor_mul(x2[:C], ta[:], sg[:])
                    cell["x2"] = x2

                def seg1():
                    xps = ps.tile([C, Nmm], F32, tag="xps")
                    nc.tensor.matmul(xps[:], wsl("1", i), cell["x2"][:],
                                     start=True, stop=True)
                    return write_out(c0, N, xps[:, :N])

                return [seg0, seg1]

            def encoder_units(s, g, is_query):
                """Chunk-level emitters for the scheduler.

                Layer 3 fuses the f0/f1 head; VQ blocks (search) are
                separate units paced by the scheduler."""
                T0h, T1 = g["T0h"], g["T1"]
                E1, O1 = g["E1"], g["O1"]
                T2, T3, T4 = g["T2"], g["T3"], g["T4"]
                d_e, d_o = (d_qe, d_qo) if is_query else (d_se, d_so)
                sfx = f"{'q' if is_query else 's'}{s}"
                st = {}

                def alloc0():
                    x0e = sb.tile([128, T0h + 8], BF16, tag=f"x0e{sfx}")
                    x0o = sb.tile([128, T0h + 8], BF16, tag=f"x0o{sfx}")
                    nc.vector.memset(x0e[:, T0h:], 0.0)
                    nc.vector.memset(x0o[:, T0h:], 0.0)
                    st.update(x0e=x0e, x0o=x0o)

                def load_dma(c0, n):
                    n = min(n, T0h - c0)
                    nc.sync.dma_start(st["x0e"][:, c0:c0 + n],
                                      d_e[s, :, c0:c0 + n])
                    nc.sync.dma_start(st["x0o"][:, c0:c0 + n],
                                      d_o[s, :, c0:c0 + n])

                def alloc_a():
                    x1e = sb.tile([128, E1 + 8], BF16, tag=f"x1e{sfx}")
                    x1o = sb.tile([128, O1 + 8], BF16, tag=f"x1o{sfx}")
                    nc.vector.memset(x1e[:, E1:], 0.0)
                    nc.vector.memset(x1o[:, O1:], 0.0)
                    nc.sync.dma_start(x1e[C:, :E1], d_z[:, :E1])
                    nc.sync.dma_start(x1o[C:, :O1], d_z[:, :O1])
                    st.update(x1e=x1e, x1o=x1o)

                def alloc_b():
                    x2f = sb.tile([128, T2 + 8], BF16, tag=f"x2f{sfx}")
                    nc.vector.memset(x2f[:, T2:], 0.0)
                    nc.sync.dma_start(x2f[C:, :T2], d_z[:, :T2])
                    x3f = sb.tile([128, T3 + 8], BF16, tag=f"x3f{sfx}")
                    nc.vector.memset(x3f[:, T3:], 0.0)
                    nc.sync.dma_start(x3f[C:, :T3], d_z[:, :T3])
                    st.update(x2f=x2f, x3f=x3f)

                def alloc_c():
                    x4f = sb.tile([128, T4 + 8], BF16, tag=f"x4f{sfx}")
                    nc.vector.memset(x4f[:, T4:], 0.0)
                    nc.sync.dma_start(x4f[C:, :T4], d_z[:, :T4])
                    # f1(relu(f0(x))); rows 80/81 of enc are const 1.0
                    # (zero weights, bias 1) for the VQ score offsets.
                    enc = sb.tile([128, T4], BF16, tag=f"enc{sfx}")
                    nc.sync.dma_start(enc[M_F1:, :], d_z[:128 - M_F1, :T4])
                    st.update(x4f=x4f, enc=enc)

                def w0(c0, N, xps):
                    # both halves on DVE: layer 0 runs in the VQ-free
                    # window where DVE idles and ACT paces the PE
                    ne, no = (N + 1) // 2, N // 2
                    h = c0 // 2
                    nc.vector.tensor_copy(
                        st["x1e"][:C, h:h + ne], xps[:, 0:N:2])
                    nc.vector.tensor_copy(
                        st["x1o"][:C, h:h + no], xps[:, 1:N:2])

                def w1(c0, N, xps):
                    nc.vector.tensor_add(
                        st["x2f"][:C, c0:c0 + N], xps,
                        st["x1o"][:C, c0 + 1:c0 + 1 + N])

                def w2(c0, N, xps):
                    nc.vector.tensor_add(
                        st["x3f"][:C, c0:c0 + N], xps,
                        st["x2f"][:C, c0 + 3:c0 + 3 + N])

                def w3(c0, N, xps):
                    x4f, x3f, enc = st["x4f"], st["x3f"], st["enc"]
                    nc.vector.tensor_add(
                        x4f[:C, c0:c0 + N], xps,
                        x3f[:C, c0 + 3:c0 + 3 + N])
                    cf = {}

                    def seg_f0():
                        p0 = ps.tile([C, N], F32, tag="xps")
                        nc.tensor.matmul(p0[:], wsl("f0", 0),
                                         x4f[:, c0:c0 + N],
                                         start=True, stop=True)
                        xf = sb.tile([128, N], BF16, tag="xf", bufs=4)
                        nc.scalar.activation(xf[:C], p0[:], AF.Relu,
                                             bias=bap("f0"))
                        cf["xf"] = xf

                    def seg_f1():
                        p1 = ps.tile([M_F1, N], F32, tag="xps")
                        nc.tensor.matmul(p1[:], wpk[:, 2960:2960 + M_F1],
                                         cf["xf"][:], start=True, stop=True)
                        nc.scalar.activation(enc[:M_F1, c0:c0 + N], p1[:],
                                             AF.Identity, bias=bap("f1"))

                    return [seg_f0, seg_f1]

                Ts = [T1, T2, T3, T4]
                writers = [w0, w1, w2, w3]

                def taps(i):
                    if i == 0:
                        return [(st["x0e"], 0), (st["x0o"], 0),
                                (st["x0e"], 1), (st["x0o"], 1)]
                    if i == 1:
                        return [(st["x1e"], 0), (st["x1o"], 0),
                                (st["x1e"], 1), (st["x1o"], 1)]
                    x = st["x2f"] if i == 2 else st["x3f"]
                    return [(x, 0), (x, 1), (x, 2), (x, 3)]

                def emit_segs(i, c):
                    return wide_chunk_segs(i, taps(i), Ts[i], writers[i],
                                           c * CH)

                nch = [-(-t // CH) for t in Ts]
                ins = [E1, T2, T3]

                def req(i, c):
                    """Chunks of layer i-1 needed before chunk c of layer i."""
                    per = CH // 2 if i == 1 else CH
                    need = min(CH * c + 515, ins[i - 1])
                    return min(nch[i - 1], -(-need // per))

                return dict(alloc0=alloc0, load_dma=load_dma, alloc_a=alloc_a,
                            alloc_b=alloc_b, alloc_c=alloc_c,
                            emit_segs=emit_segs, nch=nch, req=req, st=st)

            def run_sched(encs, prios, vqs=None, post_l3=None):
                """Software-pipelined emission: each conv unit is a list
                of PE segments (taps / 1x1 / f0 / f1); a segment is only
                emitted one turn after its predecessor so another unit's
                taps fill the PE queue between dependent matmuls.  VQ
                (matmul, max-reduce) singles are paced in between."""
                sprog = [[0] * 4 for _ in encs]   # units started
                wprog = [[0] * 4 for _ in encs]   # units fully emitted
                conv_rem = sum(sum(e["nch"]) for e in encs)
                segq = []                          # [closures, sidx, i, turn]
                vprog = {sidx: 0 for sidx in (vqs or {})}
                vtot = {sidx: 3 * v["nblk"] for sidx, v in (vqs or {}).items()}
                turn = 0

                def unit_done(sidx, i):
                    wprog[sidx][i] += 1
                    if (i == 3 and wprog[sidx][3] == encs[sidx]["nch"][3]
                            and post_l3):
                        post_l3(sidx)

                def run_seg(entry):
                    closures, sidx, i, _ = entry
                    more = closures.pop(0)()
                    if more:
                        closures.extend(more)
                    if closures:
                        segq.append([closures, sidx, i, turn])
                    else:
                        unit_done(sidx, i)

                def try_new(group):
                    nonlocal conv_rem
                    for k in range(len(group)):
                        sidx = group[(turn + k) % len(group)]
                        e, p = encs[sidx], sprog[sidx]
                        for i in (3, 2, 1, 0):
                            if p[i] >= e["nch"][i]:
                                continue
                            if i > 0 and wprog[sidx][i - 1] < e["req"](i, p[i]):
                                continue
                            segs = e["emit_segs"](i, p[i])
                            p[i] += 1
                            conv_rem -= 1
                            run_seg([segs, sidx, i, turn])
                            return True
                    return False

                def emit_vq(budget):
                    done = 0
                    keys = sorted(vqs)
                    for k in range(len(keys)):
                        if done >= budget:
                            break
                        sidx = keys[(turn + k) % len(keys)]
                        v = vqs[sidx]
                        while vprog[sidx] < vtot[sidx] and done < budget:
                            b, ti = divmod(vprog[sidx], 3)
                            if wprog[sidx][3] < v["req"](b):
                                break
                            v["emit_one"](b, ti)
                            vprog[sidx] += 1
                            done += 1
                    return done

                while (conv_rem or segq or
                       any(vprog[s] < vtot[s] for s in vprog)):
                    emitted = 0
                    # cap in-flight multi-segment units so the ring
                    # buffers (depth 4) can never cycle-deadlock
                    if len(segq) < 3:
                        for group in prios:
                            if try_new(group):
                                emitted += 1
                                break
                    if segq and segq[0][3] < turn:
                        run_seg(segq.pop(0))
                        emitted += 1
                        # drain backlogged segments faster at the conv tail
                        if len(segq) >= 2 and segq[0][3] < turn:
                            run_seg(segq.pop(0))
                            emitted += 1
                    if vqs:
                        emitted += emit_vq(3 if emitted else 8)
                    if not emitted and segq:
                        run_seg(segq.pop(0))
                        emitted = 1
                    turn += 1
                    assert emitted, "scheduler deadlock"

            # ---- encoders: query inputs first; search streams behind and
            # runs in the middle; query sample 1 is held back to overlap
            # the VQ drain at the end ----
            qencs = [encoder_units(s, GEO_QUERY, True) for s in range(SPC)]
            sencs = [encoder_units(s, GEO_SEARCH, False) for s in range(SPC)]
            for e in qencs + sencs:
                e["alloc0"]()
            for c0 in (0, 512):
                for e in qencs:
                    e["load_dma"](c0, 512)
            for e in qencs:
                e["alloc_a"]()
                e["alloc_b"]()
                e["alloc_c"]()
            # remaining static tables arrive behind the query inputs
            nc.sync.dma_start(wpk[:, LBLK:], d_wpk[:, LBLK:])
            epk3 = sb.tile([128, 3 * NK], BF16, tag="epk3")
            nc.sync.dma_start(epk3[:], d_epk3[:])
            wlt = sb.tile([128, 2], BF16, tag="wlt")
            nc.sync.dma_start(wlt[:], d_wlt[:])
            brow = sb.tile([1, 4], F32, tag="brow")
            nc.sync.dma_start(brow[:], d_blt[:])
            ztail = sb.tile([2, 32], F32, tag="ztail")
            nc.sync.dma_start(ztail[:], d_ztail[:])
            # search inputs interleaved with their zero-fills, ordered so
            # each layer's zeros land before that layer can be scheduled
            for e in sencs:
                e["load_dma"](0, 1024)
            for e in sencs:
                e["alloc_a"]()
            for e in sencs:
                e["load_dma"](1024, 1024)
            for e in sencs:
                e["alloc_b"]()
            for e in sencs:
                e["load_dma"](2048, 1024)
            for e in sencs:
                e["alloc_c"]()
            for e in sencs:
                e["load_dma"](3072, 1024)

            T4s = GEO_SEARCH["T4"]
            accs = {}
            for s in range(SPC):
                mt = sb.tile([128, 16], F32, tag=f"mt{s}")
                u0t = sb.tile([128, 16], F32, tag=f"u0t{s}")
                u1t = sb.tile([128, 16], F32, tag=f"u1t{s}")
                nc.vector.memset(mt[:], NEG)
                nc.vector.memset(u0t[:], NEG)
                nc.vector.memset(u1t[:], NEG)
                accs[s] = (mt, u0t, u1t)

            import concourse.bass_isa as bass_isa
            # warm the Q7 IRAM for partition_all_reduce early so the
            # final reduce doesn't pay the ~6us ext-isa load at the tail
            zwarm = sb.tile([128, 16], F32, tag="zwarm")
            nc.gpsimd.partition_all_reduce(zwarm[:], accs[0][0][:],
                                           channels=128,
                                           reduce_op=bass_isa.ReduceOp.max)

            zred = sb.tile([128, 4], F32, tag="zred")
            vts = {}

            def emit_vpath(s):
                """v = w_lin @ enc_q, tiled x4 through DRAM into [128,16]."""
                enc_q = qencs[s]["st"]["enc"]
                vps = ps.tile([128, NK], F32, tag="vq")
                nc.tensor.matmul(vps[:2, :504], wlt[:], enc_q[:, :504],
                                 start=True, stop=True)
                vrow = sb.tile([2, 504], F32, tag="vrow", bufs=2)
                nc.scalar.activation(vrow[:], vps[:2, :504], AF.Copy)
                for k in range(4):
                    nc.sync.dma_start(
                        d_zbuf[2 * s:2 * s + 2, 504 * k:504 * (k + 1)],
                        vrow[:])
                nc.sync.dma_start(d_zbuf[2 * s:2 * s + 2, 2016:], ztail[:])
                vt0 = sb.tile([128, 16], F32, tag=f"vt{2 * s}")
                vt1 = sb.tile([128, 16], F32, tag=f"vt{2 * s + 1}")
                nc.sync.dma_start(
                    vt0[:], d_zbuf[2 * s].rearrange("(b p) -> p b", p=128))
                nc.sync.dma_start(
                    vt1[:], d_zbuf[2 * s + 1].rearrange("(b p) -> p b", p=128))
                vts[s] = (vt0, vt1)

            def make_vq(s):
                tgts = accs[s]
                st = sencs[s]["st"]

                def emit_one(b, ti):
                    t0 = 128 * b
                    P = min(128, T4s - t0)
                    sps = ps.tile([128, NK], F32, tag="vq")
                    nc.tensor.matmul(
                        sps[:P, :], st["enc"][:, t0:t0 + P],
                        epk3[:, NK * ti:NK * (ti + 1)],
                        start=True, stop=True)
                    nc.vector.tensor_reduce(
                        tgts[ti][:P, b:b + 1], sps[:P, :], axis=AX.X,
                        op=OP.max)

                def req(b):
                    return min(sencs[s]["nch"][3], -(-(128 * (b + 1)) // CH))

                return dict(emit_one=emit_one, req=req, nblk=16)

            def post_l3(sidx):
                if sidx < SPC:          # query sample finished its encoder
                    emit_vpath(sidx)

            vqs = {SPC + s: make_vq(s) for s in range(SPC)}
            run_sched(qencs + sencs,
                      prios=[[0], [2, 3], [1]], vqs=vqs, post_l3=post_l3)

            # ---- z = (u - m)*BIG + vt, reduce ----
            for s in range(SPC):
                mt, u0t, u1t = accs[s]
                for j, ut in ((0, u0t), (1, u1t)):
                    zt = sb.tile([128, 16], F32, tag="zt", bufs=2)
                    nc.vector.tensor_sub(zt[:], ut[:], mt[:])
                    nc.vector.scalar_tensor_tensor(
                        out=zt[:], in0=zt[:], scalar=BIG, in1=vts[s][j][:],
                        op0=OP.mult, op1=OP.add)
                    nc.vector.tensor_reduce(
                        zred[:, 2 * s + j:2 * s + j + 1], zt[:], axis=AX.X,
                        op=OP.max)

            zar = sb.tile([128, 4], F32, tag="zar")
            nc.gpsimd.partition_all_reduce(zar[:], zred[:], channels=128,
                                           reduce_op=bass_isa.ReduceOp.max)
            zrow = sb.tile([1, 4], F32, tag="zrow")
            nc.vector.tensor_add(zrow[:], zar[0:1, :], brow[:])
            outv = sb.tile([1, 4], F32, tag="outv")
            nc.scalar.activation(outv[:], zrow[:], AF.Tanh)
            nc.sync.dma_start(d_out[:], outv[:])

    nc.finalize()
    return nc


_NC_CACHE = None


def _get_nc():
    global _NC_CACHE
    if _NC_CACHE is None:
        _NC_CACHE = _build()
    return _NC_CACHE


def prep_inputs(search, query, w_wide, b_wide, w_1x1, b_1x1, w_f0, b_f0,
                w_f1, b_f1, embedding, w_lin, b_lin):
    """Host-side packing -> list of per-core input maps (bf16 operands)."""
    f = np.float32
    bf = ml_dtypes.bfloat16
    search = np.asarray(search, f)
    query = np.asarray(query, f)

    def pad128(a):
        # (N, T, C) -> (N, 128, T) channel-major, rows C..127 zero
        n, t, _ = a.shape
        out = np.zeros((n, 128, t), dtype=bf)
        out[:, :C, :] = a.transpose(0, 2, 1).astype(bf)
        return out

    se = pad128(search[:, 0::2, :])
    so = pad128(search[:, 1::2, :])
    qe = pad128(query[:, 0::2, :])
    qo = pad128(query[:, 1::2, :])

    w_wide = np.asarray(w_wide, f)
    w_1x1 = np.asarray(w_1x1, f)
    b_wide = np.asarray(b_wide, f)
    b_1x1 = np.asarray(b_1x1, f)
    cols = []
    for i in range(4):
        for j in range(4):
            cols.append(w_wide[i, :C, :, j].T)     # a taps
        for j in range(4):
            cols.append(w_wide[i, C:, :, j].T)     # g taps
        cols.append(w_1x1[i, :, :, 0].T)
    cols.append(np.asarray(w_f0, f)[:, :, 0].T)
    wf1 = np.zeros((C, M_F1), f)
    wf1[:, :C] = np.asarray(w_f1, f)[:, :, 0].T   # cols 80/81 stay zero
    cols.append(wf1)
    wpk80 = np.ascontiguousarray(np.concatenate(cols, axis=1))
    assert wpk80.shape == (C, WPACK_COLS)
    wpk = np.zeros((128, WPACK_COLS), dtype=bf)
    wpk[:C] = wpk80.astype(bf)

    # cumulative 1x1 biases folded into next layer's tap biases / f0 bias
    cum = np.cumsum(b_1x1, axis=0)                 # cum[i] = b0+..+bi
    bcols = []
    for i in range(4):                             # a-tap biases
        ba = b_wide[i, :C].copy()
        if i > 0:
            ba += w_wide[i, :C].sum(axis=2) @ cum[i - 1]
        bcols.append(ba)
    for i in range(4):                             # g-tap biases
        bg = b_wide[i, C:].copy()
        if i > 0:
            bg += w_wide[i, C:].sum(axis=2) @ cum[i - 1]
        bcols.append(bg)
    bcols += [np.zeros(C, f)] * 4                  # (1x1 biases folded away)
    bf0 = np.asarray(b_f0, f) + np.asarray(w_f0, f)[:, :, 0] @ cum[3]
    bcols += [bf0, np.asarray(b_f1, f)]
    bpk = np.zeros((M_F1, 14), f)
    bpk[:C] = np.stack(bcols, axis=1)
    bpk[C, _b_off("f1")] = 1.0     # f1 rows 80/81 = 0*x + 1.0 -> const-1
    bpk[C + 1, _b_off("f1")] = 1.0

    emb = np.asarray(embedding, f)[0]            # (512, 80)
    e2 = (emb.astype(np.float64) ** 2).sum(1)
    ew = (emb.astype(np.float64) @ np.asarray(w_lin, f).T.astype(np.float64))
    epk3 = np.zeros((128, 3 * NK), f)
    for ti in range(3):
        epk3[:C, NK * ti:NK * (ti + 1)] = emb.T
        epk3[C, NK * ti:NK * (ti + 1)] = -0.5 * e2
    epk3[C + 1, NK:2 * NK] = ew[:, 0] / BIG
    epk3[C + 1, 2 * NK:3 * NK] = ew[:, 1] / BIG
    epk3 = epk3.astype(bf)
    z48 = np.zeros((48, 4104), dtype=bf)
    ztail = np.zeros((2, 32), f)
    ztail[:, 24:] = NEG
    wlt = np.zeros((128, 2), dtype=bf)
    wlt[:C] = np.ascontiguousarray(np.asarray(w_lin, f).T).astype(bf)
    b_lin = np.asarray(b_lin, f)
    blt = np.array([[b_lin[0], b_lin[1], b_lin[0], b_lin[1]]], f)

    maps = []
    for c in range(NCORES):
        sl = slice(SPC * c, SPC * (c + 1))
        maps.append({
            "se": se[sl], "so": so[sl], "qe": qe[sl], "qo": qo[sl],
            "wpk": wpk, "bpk": bpk, "epk3": epk3, "wlt": wlt, "blt": blt,
            "z48": z48, "ztail": ztail,
        })
    return maps


def kernel(**inputs):
    nc = _get_nc()
    maps = prep_inputs(**inputs)
    res = run_bass_kernel_spmd(nc, maps, core_ids=list(range(NCORES)))
    out = np.concatenate([r["out"].reshape(SPC, 2) for r in res.results],
                         axis=0)
    return out.astype(np.float32)


if __name__ == "__main__":
    import reference
    inputs = {k: np.asarray(v) for k, v in reference.setup_inputs().items()}
    got = kernel(**inputs)
    print(got)
